# revision 17
# baseline (speedup 1.0000x reference)
"""BasicRGCN Trainium2 kernel — 8-core SPMD Bass/Tile implementation.

Model (PyG-style RGCNConv x2 + global_mean_pool):
  h1 = relu(x @ root1 + b1 + sum_r mean_r(x_src) @ W1[r])
  h2 = relu(h1 @ root2 + b2 + sum_r mean_r(h1_src) @ W2[r])
  out[g] = mean over nodes in graph g of h2            -> [64, 128] f32

Distribution: nodes (and their incoming edges) are sharded over 8 cores by
destination id (12544 nodes/core). Per-relation weights are replicated.
Layer-1 edge features (15-dim x rows, scaled by 1/deg) are pre-gathered on
the host as part of input sharding. Layer-2 features (h1, device-computed)
are exchanged with a chunked AllGather (overlapped under layer-1 compute)
and gathered on-device via the batched gpsimd dma_gather custom op (int16
indices -> the h1 table is split into 4 quarters; slot groups are packed
per (dst-window, src-quarter) so each gather instruction covers one
quarter's groups of one output tile, on its own SWDGE queue).
Aggregation uses a one-hot matmul: for each 128-edge group, a bf16
selection matrix built on the vector engine (iota + is_equal against
relation-folded window keys) scatters gathered rows into per-window PSUM
accumulators on the tensor engine, which also handles duplicate
destinations for free.
"""
import sys
sys.path.insert(0, "/opt/trn_rl_repo")
import numpy as np

import concourse.bass as bass
import concourse.mybir as mybir
import concourse.tile as tile_mod
from concourse.tile import TileContext
from concourse.bacc import Bacc
from concourse.ap import AP
from concourse.masks import make_identity
from concourse.tile_rust import add_dep_helper

# ---------------------------------------------------------------- constants
NCORES = 8
N = 100000
NPAD = 100352            # 8 * 12544
PC = NPAD // NCORES      # 12544 nodes per core
W = 32                   # dst window width (4W = 128 one-hot columns)
NW = PC // W             # 392 windows per core
TW = 256 // W            # 8 windows per output tile (256 nodes)
NT = NW // TW            # 49 output tiles
H = 128                  # hidden dim
F1 = 16                  # padded layer-1 input dim (15 real)
R = 4                    # relations
NGRAPH = 64
NQ = 4                   # src table quarters (dma_gather int16 index range)
QS = NPAD // NQ          # 25088 rows per quarter
NCHUNK = 7               # AllGather chunks
CH = PC // NCHUNK        # 1792 rows per chunk
CHT = NT // NCHUNK       # 7 tiles per chunk
CHUNK_AG = False         # chunked AllGather (overlap with L1) vs single
NQUEUES = 1              # SWDGE queues used for dma_gather (1..4)
GBMAX = 8                # max groups (1024 idxs) per dma_gather: the SWDGE
                         # descriptor ring holds 128 descs/engine; a single
                         # DMA needing >127 descs/engine is illegal

_bf16 = mybir.dt.bfloat16
_f32 = mybir.dt.float32
_i16 = mybir.dt.int16


def _to_bf16(a):
    """f32 -> bf16 (round-to-nearest-even) stored as numpy uint16 view array."""
    import ml_dtypes
    return a.astype(ml_dtypes.bfloat16)


# ------------------------------------------------------- tile/walrus patches
def _patch_tile_drain():
    """This deployment's walrus accepts only ONE sync-wait per instruction:
    split the end-of-TileContext drain into single-wait drains."""
    def _patched(self, tick_clock, wait_clock):
        nc = self.nc
        drain_inst = nc.sync.drain()
        wait_clock.add_sem_waits(
            drain_inst.ins, tile_mod.ScopedClock({None: tick_clock.global_clock})
        )
        si = drain_inst.ins.sync_info
        if si is not None and si.on_wait and len(si.on_wait) > 1:
            waits = list(si.on_wait)
            si.on_wait = waits[:1]
            for i in range(1, len(waits)):
                extra = nc.sync.drain()
                esi = extra.ins.sync_info
                if esi is None:
                    extra.ins.sync_info = mybir.SyncInfo(
                        on_wait=[waits[i]], on_update=[])
                else:
                    esi.on_wait = [waits[i]]
        nc.all_engine_barrier()
        assert self.sems is not None
        popped = nc._tile_sem_poison_stack.pop()
        assert popped is self._sem_poison
        nc.clear_and_free_semaphores(list(self.sems.allocated().values()))
        nc.all_engine_barrier()
    TileContext._drain_and_barrier = _patched


_patch_tile_drain()
_legal_ctr = [0]


def _legalize_waits(nc, maxw=1):
    """Split >maxw sync-waits on any instruction onto preceding same-engine
    NoOps (engine streams are in-order, so this is semantics-preserving)."""
    for f in nc.m.functions:
        for blk in f.blocks:
            insts = list(blk.instructions)
            out = []
            changed = False
            for ins in insts:
                si = ins.sync_info
                if si is not None and si.on_wait and len(si.on_wait) > maxw:
                    waits = list(si.on_wait)
                    for i in range(0, len(waits) - maxw, maxw):
                        _legal_ctr[0] += 1
                        nop = mybir.InstNoOp(
                            name=f"legalw-{_legal_ctr[0]}", ins=[], outs=[])
                        nop.engine = ins.engine
                        nop.sync_info = mybir.SyncInfo(
                            on_wait=waits[i:i + maxw], on_update=[])
                        out.append(nop)
                    si.on_wait = waits[len(waits) - maxw:]
                    changed = True
                out.append(ins)
            if changed:
                blk.instructions = out


# ------------------------------------------------------------- group layout
def _group_layout(cap_wq):
    """Group ordering: tile t -> quarter q -> window w -> j.
    Returns (seqs, tq, tile_g0, wq_goff, TOTAL_G):
      seqs[t]   = [(local_g, w_in_tile, q), ...]
      tq[t]     = [(q, local_g0, ngroups), ...]   gather calls for the tile
      tile_g0   = [NT+1] global group offset per tile
      wq_goff   = [NW, NQ] global group offset of cell (w, q)
    """
    seqs, tq = [], []
    tile_g0 = np.zeros(NT + 1, dtype=np.int64)
    wq_goff = np.zeros((NW, NQ), dtype=np.int64)
    g = 0
    for t in range(NT):
        tile_g0[t] = g
        seq, tqr = [], []
        for q in range(NQ):
            lg0 = g - tile_g0[t]
            for wi in range(TW):
                w = t * TW + wi
                wq_goff[w, q] = g
                for _ in range(int(cap_wq[w, q])):
                    seq.append((int(g - tile_g0[t]), wi, q))
                    g += 1
            ng = (g - tile_g0[t]) - lg0
            if ng:
                tqr.append((q, int(lg0), int(ng)))
        # matmuls must run window-major: a PSUM bank can only have ONE open
        # accumulation group at a time (the slot/gather layout stays
        # quarter-major; only the emission order changes)
        seq.sort(key=lambda e: e[1])
        seqs.append(seq)
        tq.append(tqr)
    tile_g0[NT] = g
    return seqs, tq, tile_g0, wq_goff, int(g)


def _table_row(node):
    """Global node id -> h1 table row (chunk-major when CHUNK_AG)."""
    if not CHUNK_AG:
        return node
    k = node // PC
    r = node % PC
    c = r // CH
    rr = r % CH
    return c * (NCORES * CH) + k * CH + rr


# ------------------------------------------------------------- host prep
def _host_prep(x, W1, root1, b1, W2, root2, b2, edge_index, edge_type, batch):
    """Shard/repack all inputs. Returns (per_core_inmaps, host_ctx)."""
    src = np.asarray(edge_index[0], dtype=np.int64)
    dst = np.asarray(edge_index[1], dtype=np.int64)
    rel = np.asarray(edge_type, dtype=np.int64)
    batch = np.asarray(batch, dtype=np.int64)
    x = np.asarray(x, dtype=np.float32)
    E = src.shape[0]

    # per-(relation, dst) in-degree counts -> mean scale
    cnt = np.zeros((R, N), dtype=np.int64)
    np.add.at(cnt, (rel, dst), 1)
    recip = (1.0 / np.maximum(cnt, 1)).astype(np.float32)   # [R, N]

    core_of = dst // PC
    woff = dst % PC
    win = woff // W
    key = rel * W + (woff % W)                               # [0, 4W)
    trow = _table_row(src)                                   # h1 table row
    quar = trow // QS
    qrel = (trow - quar * QS).astype(np.int16)               # [0, QS)

    # per-(core, window, quarter) counts -> shared capacities
    cwq = np.zeros((NCORES, NW, NQ), dtype=np.int64)
    np.add.at(cwq, (core_of, win, quar), 1)
    cap_wq = np.ceil(cwq.max(axis=0) / 128).astype(np.int64)  # [NW, NQ]

    seqs, tq, tile_g0, wq_goff, TOTAL_G = _group_layout(cap_wq)
    NSLOT = TOTAL_G * 128

    # slot assignment: sort edges by (core, window, quarter); edges of a
    # cell fill slots wq_goff[w,q]*128 ... in order
    order = np.lexsort((quar, win, core_of))
    s_src, s_rel, s_dst = src[order], rel[order], dst[order]
    s_core, s_win, s_quar = core_of[order], win[order], quar[order]
    s_key = key[order]
    s_qrel = qrel[order]
    s_scale = recip[s_rel, s_dst].astype(np.float32)

    cell_id = (s_core * NW + s_win) * NQ + s_quar
    cell_start = np.zeros(NCORES * NW * NQ + 1, dtype=np.int64)
    np.add.at(cell_start, cell_id + 1, 1)
    cell_start = np.cumsum(cell_start)
    pos_in_cell = np.arange(E) - cell_start[cell_id]
    slot = (wq_goff[s_win, s_quar] * 128 + pos_in_cell).astype(np.int64)

    keys_all = np.full((NCORES, NSLOT), -1.0, dtype=np.float32)
    idx_all = np.zeros((NCORES, NSLOT), dtype=np.int16)      # pad -> row 0
    xsl_all = np.zeros((NCORES, NSLOT, F1), dtype=np.float32)
    keys_all[s_core, slot] = s_key
    idx_all[s_core, slot] = s_qrel
    xsl_all[s_core, slot, :15] = x[s_src] * s_scale[:, None]

    # [core, 128, TOTAL_G] layouts (slot = g*128 + p  ->  [p, g])
    keys_pg = _to_bf16(
        keys_all.reshape(NCORES, TOTAL_G, 128).transpose(0, 2, 1).copy())
    xsl_pg = _to_bf16(
        xsl_all.reshape(NCORES, TOTAL_G, 128, F1).transpose(0, 2, 1, 3).copy())
    # int16 idx, 16-partition wrap replicated to 128: [p, j] = idx[j*16+p%16]
    idx16 = idx_all.reshape(NCORES, TOTAL_G * 8, 16).transpose(0, 2, 1)
    idx16 = np.tile(idx16, (1, 8, 1)).copy()                 # [core, 128, G*8]

    # scale columns for layer 2 (partition-replicated): col (w, r*W+o)
    sc = np.zeros((NCORES, NW * 4 * W), dtype=np.float32)
    for k in range(NCORES):
        r_grid = np.repeat(np.arange(R), W)  # [4W] -> r
        o_grid = np.tile(np.arange(W), R)    # [4W] -> offset
        nodes_w = (k * PC + (np.arange(NW)[:, None] * W + o_grid[None, :]))
        nodes_w = np.minimum(nodes_w, N - 1)
        sc[k] = recip[r_grid[None, :], nodes_w].reshape(-1)
    sc_bf = _to_bf16(sc)                     # [NCORES, NW*128]
    sc_rep = np.broadcast_to(
        sc_bf[:, None, :], (NCORES, 128, NW * 4 * W)).copy()

    # batch one-hot [PC, 64] per core
    bone = np.zeros((NCORES, PC, NGRAPH), dtype=np.float32)
    for k in range(NCORES):
        nd = k * PC + np.arange(PC)
        real = nd < N
        bone[k, real, batch[nd[real]]] = 1.0
    bone_bf = _to_bf16(bone)

    # x^T own block, padded feat rows [128, PC]
    xT = np.zeros((NCORES, 128, PC), dtype=np.float32)
    for k in range(NCORES):
        nd = k * PC + np.arange(PC)
        real = nd < N
        xT[k][:15][:, real] = x[nd[real]].T
    xT_bf = _to_bf16(xT)

    # weights (replicated)
    def padw(w, rows):
        out = np.zeros((128, H), dtype=np.float32)
        out[:rows] = w
        return _to_bf16(out)

    W1p = np.stack([padw(np.asarray(W1)[r], 15) for r in range(R)])
    root1p = padw(np.asarray(root1), 15)
    W2p = np.stack([padw(np.asarray(W2)[r], H) for r in range(R)])
    root2p = padw(np.asarray(root2), H)
    b1f = np.asarray(b1, dtype=np.float32).reshape(H, 1)
    b2f = np.asarray(b2, dtype=np.float32).reshape(H, 1)

    in_maps = []
    for k in range(NCORES):
        in_maps.append({
            "xsl": xsl_pg[k],          # [128, TOTAL_G, F1] bf16
            "keys": keys_pg[k],        # [128, TOTAL_G] bf16
            "idx16": idx16[k],         # [128, TOTAL_G*8] int16
            "screp": sc_rep[k],        # [128, NW*128] bf16
            "bone": bone_bf[k],        # [PC, 64] bf16
            "xT": xT_bf[k],            # [128, PC] bf16
            "W1p": W1p, "root1p": root1p,
            "W2p": W2p, "root2p": root2p,
            "b1": b1f, "b2": b2f,
        })

    gcounts = np.maximum(np.bincount(batch, minlength=NGRAPH), 1).astype(np.float32)
    host_ctx = {"cap_wq": cap_wq, "TOTAL_G": TOTAL_G, "gcounts": gcounts}
    return in_maps, host_ctx


# ------------------------------------------------------------- device build
def _build_nc(cap_wq, legalize=None):
    import os as _os
    if legalize is None:
        legalize = not _os.environ.get("BASS_NO_LEGALIZE")
    seqs, tq, tile_g0, wq_goff, TOTAL_G = _group_layout(cap_wq)
    GTMAX = int(max(tile_g0[t + 1] - tile_g0[t] for t in range(NT)))

    nc = Bacc("TRN2", num_devices=NCORES, num_swdge_queues=NQUEUES)
    xsl = nc.dram_tensor("xsl", [128, TOTAL_G, F1], _bf16, kind="ExternalInput")
    keys = nc.dram_tensor("keys", [128, TOTAL_G], _bf16, kind="ExternalInput")
    idx16d = nc.dram_tensor("idx16", [128, TOTAL_G * 8], _i16,
                            kind="ExternalInput")
    screp = nc.dram_tensor("screp", [128, NW * 128], _bf16, kind="ExternalInput")
    bone = nc.dram_tensor("bone", [PC, NGRAPH], _bf16, kind="ExternalInput")
    xT = nc.dram_tensor("xT", [128, PC], _bf16, kind="ExternalInput")
    W1p = nc.dram_tensor("W1p", [R, 128, H], _bf16, kind="ExternalInput")
    root1p = nc.dram_tensor("root1p", [128, H], _bf16, kind="ExternalInput")
    W2p = nc.dram_tensor("W2p", [R, 128, H], _bf16, kind="ExternalInput")
    root2p = nc.dram_tensor("root2p", [128, H], _bf16, kind="ExternalInput")
    b1 = nc.dram_tensor("b1", [H, 1], _f32, kind="ExternalInput")
    b2 = nc.dram_tensor("b2", [H, 1], _f32, kind="ExternalInput")
    h1own = nc.dram_tensor("h1own", [PC, H], _bf16, kind="Internal")
    h1tab = nc.dram_tensor("h1tab", [NPAD, H], _bf16, kind="Internal",
                           addr_space="Shared")
    pool_out = nc.dram_tensor("pool_out", [NGRAPH, H], _f32, kind="ExternalOutput")

    with TileContext(nc, num_cores=NCORES) as tc:
        import contextlib
        with contextlib.ExitStack() as ctx:
            const_p = ctx.enter_context(tc.tile_pool(name="const", bufs=1))
            wpool = ctx.enter_context(tc.tile_pool(name="wts", bufs=1))
            hpool = ctx.enter_context(tc.tile_pool(name="hT", bufs=1))
            feed_p = ctx.enter_context(tc.tile_pool(name="feed", bufs=3))
            oh_p = ctx.enter_context(tc.tile_pool(name="oh", bufs=3))
            g_p = ctx.enter_context(tc.tile_pool(name="gat", bufs=3))
            sb_p = ctx.enter_context(tc.tile_pool(name="stile", bufs=2))
            sc_p = ctx.enter_context(tc.tile_pool(name="sctile", bufs=2))
            off_p = ctx.enter_context(tc.tile_pool(name="offt", bufs=3))
            tok_p = ctx.enter_context(tc.tile_pool(name="tok", bufs=3))
            bo_p = ctx.enter_context(tc.tile_pool(name="bo", bufs=3))
            misc_p = ctx.enter_context(tc.tile_pool(name="misc", bufs=2))
            ps_agg = ctx.enter_context(
                tc.tile_pool(name="ps_agg", bufs=1, space="PSUM"))
            ps_out = ctx.enter_context(
                tc.tile_pool(name="ps_out", bufs=1, space="PSUM"))
            ps_tr = ctx.enter_context(
                tc.tile_pool(name="ps_tr", bufs=2, space="PSUM"))
            ps_pool = ctx.enter_context(
                tc.tile_pool(name="ps_pool", bufs=1, space="PSUM"))

            # constants
            iota_i = const_p.tile([128, 128], mybir.dt.int32)
            nc.gpsimd.iota(iota_i[:], pattern=[[1, 128]],
                           base=0, channel_multiplier=0)
            iota_bf = const_p.tile([128, 128], _bf16)
            nc.vector.tensor_copy(iota_bf[:], iota_i[:])
            ident = const_p.tile([128, 128], _bf16)
            make_identity(nc, ident[:])

            # weights resident in SBUF
            w1t = [wpool.tile([128, H], _bf16, tag=f"w1_{r}", name=f"w1_{r}")
                   for r in range(R)]
            w2t = [wpool.tile([128, H], _bf16, tag=f"w2_{r}", name=f"w2_{r}")
                   for r in range(R)]
            r1t = wpool.tile([128, H], _bf16, tag="r1")
            r2t = wpool.tile([128, H], _bf16, tag="r2")
            b1t = wpool.tile([H, 1], _f32, tag="b1")
            b2t = wpool.tile([H, 1], _f32, tag="b2")
            for r in range(R):
                nc.sync.dma_start(out=w1t[r][:], in_=W1p[r])
                nc.sync.dma_start(out=w2t[r][:], in_=W2p[r])
            nc.sync.dma_start(out=r1t[:], in_=root1p[:, :])
            nc.sync.dma_start(out=r2t[:], in_=root2p[:, :])
            nc.sync.dma_start(out=b1t[:], in_=b1[:, :])
            nc.sync.dma_start(out=b2t[:], in_=b2[:, :])

            hT_x = hpool.tile([128, PC], _bf16, tag="hT_x")     # layer1 rhs
            hT_1 = hpool.tile([128, PC], _bf16, tag="hT_1")     # layer1 out
            nc.sync.dma_start(out=hT_x[:], in_=xT[:, :])

            pool_acc = const_p.tile([NGRAPH, H], _f32)
            nc.vector.memset(pool_acc[:], 0.0)

            def bcast_inner(tile_ap, ncols, inner):
                """[128, ncols] -> AP [128, ncols, inner] (step-0 inner)."""
                base = tile_ap
                newap = [list(base.ap[0]), [base.ap[-1][0], ncols], [0, inner]]
                return AP(base.tensor, base.offset, newap)

            def bcast_mid(tile_ap, nmid):
                """[128, 128] -> AP [128, nmid, 128] (step-0 middle)."""
                base = tile_ap
                newap = [list(base.ap[0]), [0, nmid], [base.ap[-1][0], 128]]
                return AP(base.tensor, base.offset, newap)

            ag_insts = []

            # per-window group totals per tile (for start/stop flags)
            def layer(L, hT_in, wts, roott, bt, hT_out, ag_deps=None):
                dep_done = [False]
                for t in range(NT):
                    g0, g1 = int(tile_g0[t]), int(tile_g0[t + 1])
                    GT = g1 - g0
                    if L == 2:
                        idxt = off_p.tile([128, GTMAX * 8], _i16, tag="idxt")
                        nc.sync.dma_start(out=idxt[:, :GT * 8],
                                          in_=idx16d[:, g0 * 8:g1 * 8])
                        gt_all = g_p.tile([128, GTMAX * H], _bf16, tag="gt")
                        for (q, qg0, qng) in tq[t]:
                            for lg0 in range(qg0, qg0 + qng, GBMAX):
                                ng = min(GBMAX, qg0 + qng - lg0)
                                gi = nc.gpsimd.dma_gather(
                                    gt_all[:, lg0 * H:
                                           (lg0 + ng) * H].rearrange(
                                               "p (g h) -> p g h", h=H),
                                    h1tab[q * QS:(q + 1) * QS, :],
                                    idxt[:, lg0 * 8:(lg0 + ng) * 8],
                                    num_idxs=ng * 128,
                                    num_idxs_reg=ng * 128,
                                    elem_size=H,
                                    elem_step=H,
                                    queue_num=q % NQUEUES)
                                if ag_deps and not dep_done[0]:
                                    for ad in ag_deps:
                                        add_dep_helper(
                                            gi.ins, ad,
                                            reason="L2 gather after AG")
                                    dep_done[0] = True
                    else:
                        feedt = feed_p.tile([128, GTMAX, F1], _bf16,
                                            tag="feed")
                        nc.sync.dma_start(
                            out=feedt[:, :GT, :],
                            in_=xsl[:, g0:g1, :])
                    keyt = misc_p.tile([128, GTMAX], _bf16, tag="keyt")
                    nc.sync.dma_start(out=keyt[:, :GT], in_=keys[:, g0:g1])
                    # one-hot for all groups of this tile in one DVE op
                    oht = oh_p.tile([128, GTMAX * 128], _bf16, tag="oht")
                    nc.vector.tensor_tensor(
                        out=oht[:, :GT * 128],
                        in0=bcast_mid(iota_bf[:], GT),
                        in1=bcast_inner(keyt[:, :GT], GT, 128),
                        op=mybir.AluOpType.is_equal)

                    # aggregation PSUM: two banks of 4 windows
                    agg = [ps_agg.tile([128, 512], _f32, tag=f"agg{h}",
                                       name=f"agg{h}") for h in range(2)]
                    totals = [int(cap_wq[t * TW + wi, :].sum())
                              for wi in range(TW)]
                    emitted = [0] * TW
                    M = H if L == 2 else F1
                    for (lg, wi, q) in seqs[t]:
                        ps = agg[wi // 4]
                        colsl = slice((wi % 4) * 128, (wi % 4) * 128 + 128)
                        if L == 2:
                            lhs = gt_all[:, lg * H:(lg + 1) * H]
                        else:
                            lhs = feedt[:, lg, :]
                        nc.tensor.matmul(
                            ps[:M, colsl],
                            lhsT=lhs,
                            rhs=oht[:, lg * 128:(lg + 1) * 128],
                            start=(emitted[wi] == 0),
                            stop=(emitted[wi] == totals[wi] - 1))
                        emitted[wi] += 1

                    # scale (L2) / plain (L1) copy PSUM -> SBUF S~^T bf16
                    KF = H if L == 2 else F1
                    st = sb_p.tile([128, TW * 128], _bf16, tag="st")
                    if L == 2:
                        sct = sc_p.tile([128, TW * 128], _bf16, tag="sct")
                        nc.sync.dma_start(
                            out=sct[:], in_=screp[:, t * TW * 128:
                                                  (t + 1) * TW * 128])
                        for h in range(2):
                            nc.vector.tensor_tensor(
                                out=st[:, h * 512:(h + 1) * 512],
                                in0=agg[h][:, :],
                                in1=sct[:, h * 512:(h + 1) * 512],
                                op=mybir.AluOpType.mult)
                    else:
                        for h in range(2):
                            nc.vector.tensor_copy(
                                st[:KF, h * 512:(h + 1) * 512],
                                agg[h][:KF, :])

                    # transform: out2^T [128, 256 nodes]
                    op_ps = ps_out.tile([128, 256], _f32, tag="ops")
                    nsl = slice(t * 256, (t + 1) * 256)
                    nc.tensor.matmul(op_ps[:, :], lhsT=roott[:KF, :],
                                     rhs=hT_in[:KF, nsl], start=True, stop=False)
                    st3 = st[:KF, :].rearrange("p (a b) -> p a b", b=128)
                    for r in range(R):
                        nc.tensor.matmul(op_ps[:, :], lhsT=wts[r][:KF, :],
                                         rhs=st3[:, :, r * W:(r + 1) * W],
                                         start=False,
                                         stop=(r == R - 1))
                    # bias + relu -> hT_out (bf16)
                    nc.scalar.activation(
                        out=hT_out[:, nsl], in_=op_ps[:, :],
                        func=mybir.ActivationFunctionType.Relu,
                        bias=bt[:], scale=1.0)

                    # transpose to token-major for h1 table / pooling
                    for half in range(2):
                        tr = ps_tr.tile([128, 128], _bf16, tag="tr")
                        nc.tensor.transpose(
                            tr[:, :],
                            hT_out[:, t * 256 + half * 128:
                                   t * 256 + (half + 1) * 128],
                            ident[:])
                        tok = tok_p.tile([128, 128], _bf16, tag="tok")
                        nc.scalar.activation(
                            out=tok[:], in_=tr[:, :],
                            func=mybir.ActivationFunctionType.Copy)
                        row0 = t * 256 + half * 128
                        if L == 1:
                            nc.sync.dma_start(
                                out=h1own[row0:row0 + 128, :], in_=tok[:])
                        else:
                            bt_t = bo_p.tile([128, NGRAPH], _bf16, tag="bt")
                            nc.sync.dma_start(
                                out=bt_t[:], in_=bone[row0:row0 + 128, :])
                            pp = ps_pool.tile([NGRAPH, H], _f32, tag="pp")
                            nc.tensor.matmul(pp[:, :], lhsT=bt_t[:],
                                             rhs=tok[:], start=True, stop=True)
                            nc.vector.tensor_tensor(
                                out=pool_acc[:], in0=pool_acc[:], in1=pp[:, :],
                                op=mybir.AluOpType.add)

                    # chunked AllGather: publish h1 rows as soon as a chunk
                    # of 7 tiles is complete (overlaps remaining L1 compute)
                    if CHUNK_AG and L == 1 and (t % CHT) == CHT - 1:
                        c = t // CHT
                        ag = nc.gpsimd.collective_compute(
                            "AllGather", mybir.AluOpType.bypass,
                            replica_groups=[list(range(NCORES))],
                            ins=[h1own[c * CH:(c + 1) * CH, :]],
                            outs=[h1tab[c * NCORES * CH:
                                        (c + 1) * NCORES * CH, :]])
                        ag_insts.append(ag.ins if hasattr(ag, "ins") else ag)

            layer(1, hT_x, w1t, r1t, b1t, hT_1)
            if not CHUNK_AG:
                ag = nc.gpsimd.collective_compute(
                    "AllGather", mybir.AluOpType.bypass,
                    replica_groups=[list(range(NCORES))],
                    ins=[h1own[:, :]], outs=[h1tab[:, :]])
                ag_insts.append(ag.ins if hasattr(ag, "ins") else ag)
            # hT_x is fully consumed by layer 1 -> reuse its SBUF for h2^T
            layer(2, hT_1, w2t, r2t, b2t, hT_x, ag_deps=ag_insts)

            nc.sync.dma_start(out=pool_out[:, :], in_=pool_acc[:])

    nc.finalize()
    if legalize:
        _legalize_waits(nc)
    return nc


# ------------------------------------------------------------- runner
_CACHE = {}


def _get_compiled(cap_wq):
    key = ("nc", tuple(cap_wq.reshape(-1).tolist()))
    if key not in _CACHE:
        import jax
        from jax.sharding import Mesh, PartitionSpec
        from jax.experimental.shard_map import shard_map
        from concourse.bass2jax import (
            _bass_exec_p, partition_id_tensor, install_neuronx_cc_hook)
        install_neuronx_cc_hook()
        nc = _build_nc(cap_wq)

        partition_name = (nc.partition_id_tensor.name
                          if nc.partition_id_tensor else None)
        in_names, out_names, out_avals = [], [], []
        for alloc in nc.m.functions[0].allocations:
            if not isinstance(alloc, mybir.MemoryLocationSet):
                continue
            name = alloc.memorylocations[0].name
            if alloc.kind == "ExternalInput":
                if name != partition_name and name != (
                        nc.dbg_addr.name if nc.dbg_addr is not None else None):
                    in_names.append(name)
            elif alloc.kind == "ExternalOutput":
                out_names.append(name)
                out_avals.append(jax.core.ShapedArray(
                    tuple(alloc.tensor_shape), mybir.dt.np(alloc.dtype)))
        n_params, n_outs = len(in_names), len(out_names)
        all_in = list(in_names) + list(out_names)
        if nc.dbg_addr is not None:
            all_in.append(nc.dbg_addr.name)
        if partition_name is not None:
            all_in.append(partition_name)

        def _body(*args):
            operands = list(args)
            if nc.dbg_addr is not None:
                operands.append(jax.numpy.zeros((1, 2), jax.numpy.uint32))
            if partition_name is not None:
                operands.append(partition_id_tensor())
            outs = _bass_exec_p.bind(
                *operands, out_avals=tuple(out_avals),
                in_names=tuple(all_in), out_names=tuple(out_names),
                lowering_input_output_aliases=(),
                sim_require_finite=False, sim_require_nnan=False, nc=nc)
            return tuple(outs)

        devices = jax.devices()[:NCORES]
        mesh = Mesh(np.asarray(devices), ("core",))
        import os as _os
        donate = (() if _os.environ.get("BASS_NO_DONATE")
                  else tuple(range(n_params, n_params + n_outs)))
        sharded = jax.jit(
            shard_map(_body, mesh=mesh,
                      in_specs=(PartitionSpec("core"),) * (n_params + n_outs),
                      out_specs=(PartitionSpec("core"),) * n_outs,
                      check_rep=False),
            donate_argnums=donate,
            keep_unused=True)
        _CACHE[key] = (sharded, in_names, out_names, out_avals, mesh)
    return _CACHE[key]


def run_device(in_maps, cap_wq):
    import jax
    sharded, in_names, out_names, out_avals, mesh = _get_compiled(cap_wq)
    concat_in = [
        np.concatenate([np.asarray(in_maps[c][name]) for c in range(NCORES)],
                       axis=0)
        for name in in_names]
    concat_zeros = [
        np.zeros((NCORES * a.shape[0], *a.shape[1:]), a.dtype)
        for a in out_avals]
    out_arrs = sharded(*concat_in, *concat_zeros)
    jax.block_until_ready(out_arrs)
    res = [
        {name: np.asarray(out_arrs[i]).reshape(NCORES, *out_avals[i].shape)[c]
         for i, name in enumerate(out_names)}
        for c in range(NCORES)]
    return res


def kernel(x, W1, root1, b1, W2, root2, b2, edge_index, edge_type, batch):
    in_maps, hc = _host_prep(x, W1, root1, b1, W2, root2, b2,
                             edge_index, edge_type, batch)
    res = run_device(in_maps, hc["cap_wq"])
    total = np.zeros((NGRAPH, H), dtype=np.float32)
    for k in range(NCORES):
        total += res[k]["pool_out"]
    return (total / hc["gcounts"][:, None]).astype(np.float32)


# revision 18
# speedup vs baseline: 1.8350x; 1.8350x over previous
"""BasicRGCN Trainium2 kernel — 8-core SPMD Bass/Tile implementation.

Model (PyG-style RGCNConv x2 + global_mean_pool):
  h1 = relu(x @ root1 + b1 + sum_r mean_r(x_src) @ W1[r])
  h2 = relu(h1 @ root2 + b2 + sum_r mean_r(h1_src) @ W2[r])
  out[g] = mean over nodes in graph g of h2            -> [64, 128] f32

Distribution: nodes (and their incoming edges) are sharded over 8 cores by
destination id (12544 nodes/core). Per-relation weights are replicated.
Layer-1 edge features (15-dim x rows, scaled by 1/deg) are pre-gathered on
the host as part of input sharding. Layer-2 features (h1, device-computed)
are exchanged with a chunked AllGather (overlapped under layer-1 compute)
and gathered on-device via the batched gpsimd dma_gather custom op (int16
indices -> the h1 table is split into 4 quarters; slot groups are packed
per (dst-window, src-quarter) so each gather instruction covers one
quarter's groups of one output tile, on its own SWDGE queue).
Aggregation uses a one-hot matmul: for each 128-edge group, a bf16
selection matrix built on the vector engine (iota + is_equal against
relation-folded window keys) scatters gathered rows into per-window PSUM
accumulators on the tensor engine, which also handles duplicate
destinations for free.
"""
import sys
sys.path.insert(0, "/opt/trn_rl_repo")
import numpy as np

import concourse.bass as bass
import concourse.mybir as mybir
import concourse.tile as tile_mod
from concourse.tile import TileContext
from concourse.bacc import Bacc
from concourse.ap import AP
from concourse.masks import make_identity
from concourse.tile_rust import add_dep_helper

# ---------------------------------------------------------------- constants
NCORES = 8
N = 100000
NPAD = 100352            # 8 * 12544
PC = NPAD // NCORES      # 12544 nodes per core
W = 32                   # dst window width (4W = 128 one-hot columns)
NW = PC // W             # 392 windows per core
TW = 256 // W            # 8 windows per output tile (256 nodes)
NT = NW // TW            # 49 output tiles
H = 128                  # hidden dim
F1 = 16                  # padded layer-1 input dim (15 real)
R = 4                    # relations
NGRAPH = 64
NQ = 4                   # src table quarters (dma_gather int16 index range)
QS = NPAD // NQ          # 25088 rows per quarter
NCHUNK = 7               # AllGather chunks
CH = PC // NCHUNK        # 1792 rows per chunk
CHT = NT // NCHUNK       # 7 tiles per chunk
CHUNK_AG = False         # chunked AllGather (overlap with L1) vs single
NQUEUES = 4              # SWDGE queues used for dma_gather (1..4)
GBMAX = 8                # max groups (1024 idxs) per dma_gather: the SWDGE
                         # descriptor ring holds 128 descs/engine; a single
                         # DMA needing >127 descs/engine is illegal

_bf16 = mybir.dt.bfloat16
_f32 = mybir.dt.float32
_i16 = mybir.dt.int16


def _to_bf16(a):
    """f32 -> bf16 (round-to-nearest-even) stored as numpy uint16 view array."""
    import ml_dtypes
    return a.astype(ml_dtypes.bfloat16)


# ------------------------------------------------------- tile/walrus patches
def _patch_tile_drain():
    """This deployment's walrus accepts only ONE sync-wait per instruction:
    split the end-of-TileContext drain into single-wait drains."""
    def _patched(self, tick_clock, wait_clock):
        nc = self.nc
        drain_inst = nc.sync.drain()
        wait_clock.add_sem_waits(
            drain_inst.ins, tile_mod.ScopedClock({None: tick_clock.global_clock})
        )
        si = drain_inst.ins.sync_info
        if si is not None and si.on_wait and len(si.on_wait) > 1:
            waits = list(si.on_wait)
            si.on_wait = waits[:1]
            for i in range(1, len(waits)):
                extra = nc.sync.drain()
                esi = extra.ins.sync_info
                if esi is None:
                    extra.ins.sync_info = mybir.SyncInfo(
                        on_wait=[waits[i]], on_update=[])
                else:
                    esi.on_wait = [waits[i]]
        nc.all_engine_barrier()
        assert self.sems is not None
        popped = nc._tile_sem_poison_stack.pop()
        assert popped is self._sem_poison
        nc.clear_and_free_semaphores(list(self.sems.allocated().values()))
        nc.all_engine_barrier()
    TileContext._drain_and_barrier = _patched


_patch_tile_drain()
_legal_ctr = [0]


def _legalize_waits(nc, maxw=1):
    """Split >maxw sync-waits on any instruction onto preceding same-engine
    NoOps (engine streams are in-order, so this is semantics-preserving)."""
    for f in nc.m.functions:
        for blk in f.blocks:
            insts = list(blk.instructions)
            out = []
            changed = False
            for ins in insts:
                si = ins.sync_info
                if si is not None and si.on_wait and len(si.on_wait) > maxw:
                    waits = list(si.on_wait)
                    for i in range(0, len(waits) - maxw, maxw):
                        _legal_ctr[0] += 1
                        nop = mybir.InstNoOp(
                            name=f"legalw-{_legal_ctr[0]}", ins=[], outs=[])
                        nop.engine = ins.engine
                        nop.sync_info = mybir.SyncInfo(
                            on_wait=waits[i:i + maxw], on_update=[])
                        out.append(nop)
                    si.on_wait = waits[len(waits) - maxw:]
                    changed = True
                out.append(ins)
            if changed:
                blk.instructions = out


# ------------------------------------------------------------- group layout
def _group_layout(cap_wq):
    """Group ordering: tile t -> quarter q -> window w -> j.
    Returns (seqs, tq, tile_g0, wq_goff, TOTAL_G):
      seqs[t]   = [(local_g, w_in_tile, q), ...]
      tq[t]     = [(q, local_g0, ngroups), ...]   gather calls for the tile
      tile_g0   = [NT+1] global group offset per tile
      wq_goff   = [NW, NQ] global group offset of cell (w, q)
    """
    seqs, tq = [], []
    tile_g0 = np.zeros(NT + 1, dtype=np.int64)
    wq_goff = np.zeros((NW, NQ), dtype=np.int64)
    g = 0
    for t in range(NT):
        tile_g0[t] = g
        seq, tqr = [], []
        for q in range(NQ):
            lg0 = g - tile_g0[t]
            for wi in range(TW):
                w = t * TW + wi
                wq_goff[w, q] = g
                for _ in range(int(cap_wq[w, q])):
                    seq.append((int(g - tile_g0[t]), wi, q))
                    g += 1
            ng = (g - tile_g0[t]) - lg0
            if ng:
                tqr.append((q, int(lg0), int(ng)))
        # matmuls must run window-major: a PSUM bank can only have ONE open
        # accumulation group at a time (the slot/gather layout stays
        # quarter-major; only the emission order changes)
        seq.sort(key=lambda e: e[1])
        seqs.append(seq)
        tq.append(tqr)
    tile_g0[NT] = g
    return seqs, tq, tile_g0, wq_goff, int(g)


def _table_row(node):
    """Global node id -> h1 table row (chunk-major when CHUNK_AG)."""
    if not CHUNK_AG:
        return node
    k = node // PC
    r = node % PC
    c = r // CH
    rr = r % CH
    return c * (NCORES * CH) + k * CH + rr


# ------------------------------------------------------------- host prep
def _host_prep(x, W1, root1, b1, W2, root2, b2, edge_index, edge_type, batch):
    """Shard/repack all inputs. Returns (per_core_inmaps, host_ctx)."""
    src = np.asarray(edge_index[0], dtype=np.int64)
    dst = np.asarray(edge_index[1], dtype=np.int64)
    rel = np.asarray(edge_type, dtype=np.int64)
    batch = np.asarray(batch, dtype=np.int64)
    x = np.asarray(x, dtype=np.float32)
    E = src.shape[0]

    # per-(relation, dst) in-degree counts -> mean scale
    cnt = np.zeros((R, N), dtype=np.int64)
    np.add.at(cnt, (rel, dst), 1)
    recip = (1.0 / np.maximum(cnt, 1)).astype(np.float32)   # [R, N]

    core_of = dst // PC
    woff = dst % PC
    win = woff // W
    key = rel * W + (woff % W)                               # [0, 4W)
    trow = _table_row(src)                                   # h1 table row
    quar = trow // QS
    qrel = (trow - quar * QS).astype(np.int16)               # [0, QS)

    # per-(core, window, quarter) counts -> shared capacities
    cwq = np.zeros((NCORES, NW, NQ), dtype=np.int64)
    np.add.at(cwq, (core_of, win, quar), 1)
    cap_wq = np.ceil(cwq.max(axis=0) / 128).astype(np.int64)  # [NW, NQ]

    seqs, tq, tile_g0, wq_goff, TOTAL_G = _group_layout(cap_wq)
    NSLOT = TOTAL_G * 128

    # slot assignment: sort edges by (core, window, quarter); edges of a
    # cell fill slots wq_goff[w,q]*128 ... in order
    order = np.lexsort((quar, win, core_of))
    s_src, s_rel, s_dst = src[order], rel[order], dst[order]
    s_core, s_win, s_quar = core_of[order], win[order], quar[order]
    s_key = key[order]
    s_qrel = qrel[order]
    s_scale = recip[s_rel, s_dst].astype(np.float32)

    cell_id = (s_core * NW + s_win) * NQ + s_quar
    cell_start = np.zeros(NCORES * NW * NQ + 1, dtype=np.int64)
    np.add.at(cell_start, cell_id + 1, 1)
    cell_start = np.cumsum(cell_start)
    pos_in_cell = np.arange(E) - cell_start[cell_id]
    slot = (wq_goff[s_win, s_quar] * 128 + pos_in_cell).astype(np.int64)

    keys_all = np.full((NCORES, NSLOT), -1.0, dtype=np.float32)
    idx_all = np.zeros((NCORES, NSLOT), dtype=np.int16)      # pad -> row 0
    xsl_all = np.zeros((NCORES, NSLOT, F1), dtype=np.float32)
    keys_all[s_core, slot] = s_key
    idx_all[s_core, slot] = s_qrel
    xsl_all[s_core, slot, :15] = x[s_src] * s_scale[:, None]

    # [core, 128, TOTAL_G] layouts (slot = g*128 + p  ->  [p, g])
    keys_pg = _to_bf16(
        keys_all.reshape(NCORES, TOTAL_G, 128).transpose(0, 2, 1).copy())
    xsl_pg = _to_bf16(
        xsl_all.reshape(NCORES, TOTAL_G, 128, F1).transpose(0, 2, 1, 3).copy())
    # int16 idx, 16-partition wrap replicated to 128: [p, j] = idx[j*16+p%16]
    idx16 = idx_all.reshape(NCORES, TOTAL_G * 8, 16).transpose(0, 2, 1)
    idx16 = np.tile(idx16, (1, 8, 1)).copy()                 # [core, 128, G*8]

    # scale columns for layer 2 (partition-replicated): col (w, r*W+o)
    sc = np.zeros((NCORES, NW * 4 * W), dtype=np.float32)
    for k in range(NCORES):
        r_grid = np.repeat(np.arange(R), W)  # [4W] -> r
        o_grid = np.tile(np.arange(W), R)    # [4W] -> offset
        nodes_w = (k * PC + (np.arange(NW)[:, None] * W + o_grid[None, :]))
        nodes_w = np.minimum(nodes_w, N - 1)
        sc[k] = recip[r_grid[None, :], nodes_w].reshape(-1)
    sc_bf = _to_bf16(sc)                     # [NCORES, NW*128]
    sc_rep = np.broadcast_to(
        sc_bf[:, None, :], (NCORES, 128, NW * 4 * W)).copy()

    # batch one-hot [PC, 64] per core
    bone = np.zeros((NCORES, PC, NGRAPH), dtype=np.float32)
    for k in range(NCORES):
        nd = k * PC + np.arange(PC)
        real = nd < N
        bone[k, real, batch[nd[real]]] = 1.0
    bone_bf = _to_bf16(bone)

    # x^T own block, padded feat rows [128, PC]
    xT = np.zeros((NCORES, 128, PC), dtype=np.float32)
    for k in range(NCORES):
        nd = k * PC + np.arange(PC)
        real = nd < N
        xT[k][:15][:, real] = x[nd[real]].T
    xT_bf = _to_bf16(xT)

    # weights (replicated)
    def padw(w, rows):
        out = np.zeros((128, H), dtype=np.float32)
        out[:rows] = w
        return _to_bf16(out)

    W1p = np.stack([padw(np.asarray(W1)[r], 15) for r in range(R)])
    root1p = padw(np.asarray(root1), 15)
    W2p = np.stack([padw(np.asarray(W2)[r], H) for r in range(R)])
    root2p = padw(np.asarray(root2), H)
    b1f = np.asarray(b1, dtype=np.float32).reshape(H, 1)
    b2f = np.asarray(b2, dtype=np.float32).reshape(H, 1)

    in_maps = []
    for k in range(NCORES):
        in_maps.append({
            "xsl": xsl_pg[k],          # [128, TOTAL_G, F1] bf16
            "keys": keys_pg[k],        # [128, TOTAL_G] bf16
            "idx16": idx16[k],         # [128, TOTAL_G*8] int16
            "screp": sc_rep[k],        # [128, NW*128] bf16
            "bone": bone_bf[k],        # [PC, 64] bf16
            "xT": xT_bf[k],            # [128, PC] bf16
            "W1p": W1p, "root1p": root1p,
            "W2p": W2p, "root2p": root2p,
            "b1": b1f, "b2": b2f,
        })

    gcounts = np.maximum(np.bincount(batch, minlength=NGRAPH), 1).astype(np.float32)
    host_ctx = {"cap_wq": cap_wq, "TOTAL_G": TOTAL_G, "gcounts": gcounts}
    return in_maps, host_ctx


# ------------------------------------------------------------- device build
def _build_nc(cap_wq, legalize=None):
    import os as _os
    if legalize is None:
        legalize = not _os.environ.get("BASS_NO_LEGALIZE")
    seqs, tq, tile_g0, wq_goff, TOTAL_G = _group_layout(cap_wq)
    GTMAX = int(max(tile_g0[t + 1] - tile_g0[t] for t in range(NT)))

    nc = Bacc("TRN2", num_devices=NCORES, num_swdge_queues=NQUEUES)
    xsl = nc.dram_tensor("xsl", [128, TOTAL_G, F1], _bf16, kind="ExternalInput")
    keys = nc.dram_tensor("keys", [128, TOTAL_G], _bf16, kind="ExternalInput")
    idx16d = nc.dram_tensor("idx16", [128, TOTAL_G * 8], _i16,
                            kind="ExternalInput")
    screp = nc.dram_tensor("screp", [128, NW * 128], _bf16, kind="ExternalInput")
    bone = nc.dram_tensor("bone", [PC, NGRAPH], _bf16, kind="ExternalInput")
    xT = nc.dram_tensor("xT", [128, PC], _bf16, kind="ExternalInput")
    W1p = nc.dram_tensor("W1p", [R, 128, H], _bf16, kind="ExternalInput")
    root1p = nc.dram_tensor("root1p", [128, H], _bf16, kind="ExternalInput")
    W2p = nc.dram_tensor("W2p", [R, 128, H], _bf16, kind="ExternalInput")
    root2p = nc.dram_tensor("root2p", [128, H], _bf16, kind="ExternalInput")
    b1 = nc.dram_tensor("b1", [H, 1], _f32, kind="ExternalInput")
    b2 = nc.dram_tensor("b2", [H, 1], _f32, kind="ExternalInput")
    h1own = nc.dram_tensor("h1own", [PC, H], _bf16, kind="Internal")
    h1tab = nc.dram_tensor("h1tab", [NPAD, H], _bf16, kind="Internal",
                           addr_space="Shared")
    pool_out = nc.dram_tensor("pool_out", [NGRAPH, H], _f32, kind="ExternalOutput")

    with TileContext(nc, num_cores=NCORES) as tc:
        import contextlib
        with contextlib.ExitStack() as ctx:
            const_p = ctx.enter_context(tc.tile_pool(name="const", bufs=1))
            wpool = ctx.enter_context(tc.tile_pool(name="wts", bufs=1))
            hpool = ctx.enter_context(tc.tile_pool(name="hT", bufs=1))
            feed_p = ctx.enter_context(tc.tile_pool(name="feed", bufs=3))
            oh_p = ctx.enter_context(tc.tile_pool(name="oh", bufs=3))
            g_p = ctx.enter_context(tc.tile_pool(name="gat", bufs=3))
            sb_p = ctx.enter_context(tc.tile_pool(name="stile", bufs=2))
            sc_p = ctx.enter_context(tc.tile_pool(name="sctile", bufs=2))
            off_p = ctx.enter_context(tc.tile_pool(name="offt", bufs=3))
            tok_p = ctx.enter_context(tc.tile_pool(name="tok", bufs=3))
            bo_p = ctx.enter_context(tc.tile_pool(name="bo", bufs=3))
            misc_p = ctx.enter_context(tc.tile_pool(name="misc", bufs=2))
            ps_agg = ctx.enter_context(
                tc.tile_pool(name="ps_agg", bufs=1, space="PSUM"))
            ps_out = ctx.enter_context(
                tc.tile_pool(name="ps_out", bufs=1, space="PSUM"))
            ps_tr = ctx.enter_context(
                tc.tile_pool(name="ps_tr", bufs=2, space="PSUM"))
            ps_pool = ctx.enter_context(
                tc.tile_pool(name="ps_pool", bufs=1, space="PSUM"))

            # constants
            iota_i = const_p.tile([128, 128], mybir.dt.int32)
            nc.gpsimd.iota(iota_i[:], pattern=[[1, 128]],
                           base=0, channel_multiplier=0)
            iota_bf = const_p.tile([128, 128], _bf16)
            nc.vector.tensor_copy(iota_bf[:], iota_i[:])
            ident = const_p.tile([128, 128], _bf16)
            make_identity(nc, ident[:])

            # weights resident in SBUF
            w1t = [wpool.tile([128, H], _bf16, tag=f"w1_{r}", name=f"w1_{r}")
                   for r in range(R)]
            w2t = [wpool.tile([128, H], _bf16, tag=f"w2_{r}", name=f"w2_{r}")
                   for r in range(R)]
            r1t = wpool.tile([128, H], _bf16, tag="r1")
            r2t = wpool.tile([128, H], _bf16, tag="r2")
            b1t = wpool.tile([H, 1], _f32, tag="b1")
            b2t = wpool.tile([H, 1], _f32, tag="b2")
            for r in range(R):
                nc.sync.dma_start(out=w1t[r][:], in_=W1p[r])
                nc.sync.dma_start(out=w2t[r][:], in_=W2p[r])
            nc.sync.dma_start(out=r1t[:], in_=root1p[:, :])
            nc.sync.dma_start(out=r2t[:], in_=root2p[:, :])
            nc.sync.dma_start(out=b1t[:], in_=b1[:, :])
            nc.sync.dma_start(out=b2t[:], in_=b2[:, :])

            hT_x = hpool.tile([128, PC], _bf16, tag="hT_x")     # layer1 rhs
            hT_1 = hpool.tile([128, PC], _bf16, tag="hT_1")     # layer1 out
            nc.sync.dma_start(out=hT_x[:], in_=xT[:, :])

            pool_acc = const_p.tile([NGRAPH, H], _f32)
            nc.vector.memset(pool_acc[:], 0.0)

            def bcast_inner(tile_ap, ncols, inner):
                """[128, ncols] -> AP [128, ncols, inner] (step-0 inner)."""
                base = tile_ap
                newap = [list(base.ap[0]), [base.ap[-1][0], ncols], [0, inner]]
                return AP(base.tensor, base.offset, newap)

            def bcast_mid(tile_ap, nmid):
                """[128, 128] -> AP [128, nmid, 128] (step-0 middle)."""
                base = tile_ap
                newap = [list(base.ap[0]), [0, nmid], [base.ap[-1][0], 128]]
                return AP(base.tensor, base.offset, newap)

            ag_insts = []

            # per-window group totals per tile (for start/stop flags)
            def layer(L, hT_in, wts, roott, bt, hT_out, ag_deps=None):
                dep_done = [False]
                for t in range(NT):
                    g0, g1 = int(tile_g0[t]), int(tile_g0[t + 1])
                    GT = g1 - g0
                    if L == 2:
                        idxt = off_p.tile([128, GTMAX * 8], _i16, tag="idxt")
                        nc.sync.dma_start(out=idxt[:, :GT * 8],
                                          in_=idx16d[:, g0 * 8:g1 * 8])
                        gt_all = g_p.tile([128, GTMAX * H], _bf16, tag="gt")
                        for (q, qg0, qng) in tq[t]:
                            for lg0 in range(qg0, qg0 + qng, GBMAX):
                                ng = min(GBMAX, qg0 + qng - lg0)
                                gi = nc.gpsimd.dma_gather(
                                    gt_all[:, lg0 * H:
                                           (lg0 + ng) * H].rearrange(
                                               "p (g h) -> p g h", h=H),
                                    h1tab[q * QS:(q + 1) * QS, :],
                                    idxt[:, lg0 * 8:(lg0 + ng) * 8],
                                    num_idxs=ng * 128,
                                    num_idxs_reg=ng * 128,
                                    elem_size=H,
                                    elem_step=H,
                                    single_packet=False,
                                    queue_num=q % NQUEUES)
                                if ag_deps and not dep_done[0]:
                                    for ad in ag_deps:
                                        add_dep_helper(
                                            gi.ins, ad,
                                            reason="L2 gather after AG")
                                    dep_done[0] = True
                    else:
                        feedt = feed_p.tile([128, GTMAX, F1], _bf16,
                                            tag="feed")
                        nc.sync.dma_start(
                            out=feedt[:, :GT, :],
                            in_=xsl[:, g0:g1, :])
                    keyt = misc_p.tile([128, GTMAX], _bf16, tag="keyt")
                    nc.sync.dma_start(out=keyt[:, :GT], in_=keys[:, g0:g1])
                    # one-hot for all groups of this tile in one DVE op
                    oht = oh_p.tile([128, GTMAX * 128], _bf16, tag="oht")
                    nc.vector.tensor_tensor(
                        out=oht[:, :GT * 128],
                        in0=bcast_mid(iota_bf[:], GT),
                        in1=bcast_inner(keyt[:, :GT], GT, 128),
                        op=mybir.AluOpType.is_equal)

                    # aggregation PSUM: two banks of 4 windows
                    agg = [ps_agg.tile([128, 512], _f32, tag=f"agg{h}",
                                       name=f"agg{h}") for h in range(2)]
                    totals = [int(cap_wq[t * TW + wi, :].sum())
                              for wi in range(TW)]
                    emitted = [0] * TW
                    M = H if L == 2 else F1
                    for (lg, wi, q) in seqs[t]:
                        ps = agg[wi // 4]
                        colsl = slice((wi % 4) * 128, (wi % 4) * 128 + 128)
                        if L == 2:
                            lhs = gt_all[:, lg * H:(lg + 1) * H]
                        else:
                            lhs = feedt[:, lg, :]
                        nc.tensor.matmul(
                            ps[:M, colsl],
                            lhsT=lhs,
                            rhs=oht[:, lg * 128:(lg + 1) * 128],
                            start=(emitted[wi] == 0),
                            stop=(emitted[wi] == totals[wi] - 1))
                        emitted[wi] += 1

                    # scale (L2) / plain (L1) copy PSUM -> SBUF S~^T bf16
                    KF = H if L == 2 else F1
                    st = sb_p.tile([128, TW * 128], _bf16, tag="st")
                    if L == 2:
                        sct = sc_p.tile([128, TW * 128], _bf16, tag="sct")
                        nc.sync.dma_start(
                            out=sct[:], in_=screp[:, t * TW * 128:
                                                  (t + 1) * TW * 128])
                        for h in range(2):
                            nc.vector.tensor_tensor(
                                out=st[:, h * 512:(h + 1) * 512],
                                in0=agg[h][:, :],
                                in1=sct[:, h * 512:(h + 1) * 512],
                                op=mybir.AluOpType.mult)
                    else:
                        for h in range(2):
                            nc.vector.tensor_copy(
                                st[:KF, h * 512:(h + 1) * 512],
                                agg[h][:KF, :])

                    # transform: out2^T [128, 256 nodes]
                    op_ps = ps_out.tile([128, 256], _f32, tag="ops")
                    nsl = slice(t * 256, (t + 1) * 256)
                    nc.tensor.matmul(op_ps[:, :], lhsT=roott[:KF, :],
                                     rhs=hT_in[:KF, nsl], start=True, stop=False)
                    st3 = st[:KF, :].rearrange("p (a b) -> p a b", b=128)
                    for r in range(R):
                        nc.tensor.matmul(op_ps[:, :], lhsT=wts[r][:KF, :],
                                         rhs=st3[:, :, r * W:(r + 1) * W],
                                         start=False,
                                         stop=(r == R - 1))
                    # bias + relu -> hT_out (bf16)
                    nc.scalar.activation(
                        out=hT_out[:, nsl], in_=op_ps[:, :],
                        func=mybir.ActivationFunctionType.Relu,
                        bias=bt[:], scale=1.0)

                    # transpose to token-major for h1 table / pooling
                    for half in range(2):
                        tr = ps_tr.tile([128, 128], _bf16, tag="tr")
                        nc.tensor.transpose(
                            tr[:, :],
                            hT_out[:, t * 256 + half * 128:
                                   t * 256 + (half + 1) * 128],
                            ident[:])
                        tok = tok_p.tile([128, 128], _bf16, tag="tok")
                        nc.scalar.activation(
                            out=tok[:], in_=tr[:, :],
                            func=mybir.ActivationFunctionType.Copy)
                        row0 = t * 256 + half * 128
                        if L == 1:
                            nc.sync.dma_start(
                                out=h1own[row0:row0 + 128, :], in_=tok[:])
                        else:
                            bt_t = bo_p.tile([128, NGRAPH], _bf16, tag="bt")
                            nc.sync.dma_start(
                                out=bt_t[:], in_=bone[row0:row0 + 128, :])
                            pp = ps_pool.tile([NGRAPH, H], _f32, tag="pp")
                            nc.tensor.matmul(pp[:, :], lhsT=bt_t[:],
                                             rhs=tok[:], start=True, stop=True)
                            nc.vector.tensor_tensor(
                                out=pool_acc[:], in0=pool_acc[:], in1=pp[:, :],
                                op=mybir.AluOpType.add)

                    # chunked AllGather: publish h1 rows as soon as a chunk
                    # of 7 tiles is complete (overlaps remaining L1 compute)
                    if CHUNK_AG and L == 1 and (t % CHT) == CHT - 1:
                        c = t // CHT
                        ag = nc.gpsimd.collective_compute(
                            "AllGather", mybir.AluOpType.bypass,
                            replica_groups=[list(range(NCORES))],
                            ins=[h1own[c * CH:(c + 1) * CH, :]],
                            outs=[h1tab[c * NCORES * CH:
                                        (c + 1) * NCORES * CH, :]])
                        ag_insts.append(ag.ins if hasattr(ag, "ins") else ag)

            layer(1, hT_x, w1t, r1t, b1t, hT_1)
            if not CHUNK_AG:
                ag = nc.gpsimd.collective_compute(
                    "AllGather", mybir.AluOpType.bypass,
                    replica_groups=[list(range(NCORES))],
                    ins=[h1own[:, :]], outs=[h1tab[:, :]])
                ag_insts.append(ag.ins if hasattr(ag, "ins") else ag)
            # hT_x is fully consumed by layer 1 -> reuse its SBUF for h2^T
            layer(2, hT_1, w2t, r2t, b2t, hT_x, ag_deps=ag_insts)

            nc.sync.dma_start(out=pool_out[:, :], in_=pool_acc[:])

    nc.finalize()
    if legalize:
        _legalize_waits(nc)
    return nc


# ------------------------------------------------------------- runner
_CACHE = {}


def _get_compiled(cap_wq):
    key = ("nc", tuple(cap_wq.reshape(-1).tolist()))
    if key not in _CACHE:
        import jax
        from jax.sharding import Mesh, PartitionSpec
        from jax.experimental.shard_map import shard_map
        from concourse.bass2jax import (
            _bass_exec_p, partition_id_tensor, install_neuronx_cc_hook)
        install_neuronx_cc_hook()
        nc = _build_nc(cap_wq)

        partition_name = (nc.partition_id_tensor.name
                          if nc.partition_id_tensor else None)
        in_names, out_names, out_avals = [], [], []
        for alloc in nc.m.functions[0].allocations:
            if not isinstance(alloc, mybir.MemoryLocationSet):
                continue
            name = alloc.memorylocations[0].name
            if alloc.kind == "ExternalInput":
                if name != partition_name and name != (
                        nc.dbg_addr.name if nc.dbg_addr is not None else None):
                    in_names.append(name)
            elif alloc.kind == "ExternalOutput":
                out_names.append(name)
                out_avals.append(jax.core.ShapedArray(
                    tuple(alloc.tensor_shape), mybir.dt.np(alloc.dtype)))
        n_params, n_outs = len(in_names), len(out_names)
        all_in = list(in_names) + list(out_names)
        if nc.dbg_addr is not None:
            all_in.append(nc.dbg_addr.name)
        if partition_name is not None:
            all_in.append(partition_name)

        def _body(*args):
            operands = list(args)
            if nc.dbg_addr is not None:
                operands.append(jax.numpy.zeros((1, 2), jax.numpy.uint32))
            if partition_name is not None:
                operands.append(partition_id_tensor())
            outs = _bass_exec_p.bind(
                *operands, out_avals=tuple(out_avals),
                in_names=tuple(all_in), out_names=tuple(out_names),
                lowering_input_output_aliases=(),
                sim_require_finite=False, sim_require_nnan=False, nc=nc)
            return tuple(outs)

        devices = jax.devices()[:NCORES]
        mesh = Mesh(np.asarray(devices), ("core",))
        import os as _os
        donate = (() if _os.environ.get("BASS_NO_DONATE")
                  else tuple(range(n_params, n_params + n_outs)))
        sharded = jax.jit(
            shard_map(_body, mesh=mesh,
                      in_specs=(PartitionSpec("core"),) * (n_params + n_outs),
                      out_specs=(PartitionSpec("core"),) * n_outs,
                      check_rep=False),
            donate_argnums=donate,
            keep_unused=True)
        _CACHE[key] = (sharded, in_names, out_names, out_avals, mesh)
    return _CACHE[key]


def run_device(in_maps, cap_wq):
    import jax
    sharded, in_names, out_names, out_avals, mesh = _get_compiled(cap_wq)
    concat_in = [
        np.concatenate([np.asarray(in_maps[c][name]) for c in range(NCORES)],
                       axis=0)
        for name in in_names]
    concat_zeros = [
        np.zeros((NCORES * a.shape[0], *a.shape[1:]), a.dtype)
        for a in out_avals]
    out_arrs = sharded(*concat_in, *concat_zeros)
    jax.block_until_ready(out_arrs)
    res = [
        {name: np.asarray(out_arrs[i]).reshape(NCORES, *out_avals[i].shape)[c]
         for i, name in enumerate(out_names)}
        for c in range(NCORES)]
    return res


def kernel(x, W1, root1, b1, W2, root2, b2, edge_index, edge_type, batch):
    in_maps, hc = _host_prep(x, W1, root1, b1, W2, root2, b2,
                             edge_index, edge_type, batch)
    res = run_device(in_maps, hc["cap_wq"])
    total = np.zeros((NGRAPH, H), dtype=np.float32)
    for k in range(NCORES):
        total += res[k]["pool_out"]
    return (total / hc["gcounts"][:, None]).astype(np.float32)


# revision 20
# speedup vs baseline: 2.0576x; 1.1214x over previous
"""BasicRGCN Trainium2 kernel — 8-core SPMD Bass/Tile implementation.

Model (PyG-style RGCNConv x2 + global_mean_pool):
  h1 = relu(x @ root1 + b1 + sum_r mean_r(x_src) @ W1[r])
  h2 = relu(h1 @ root2 + b2 + sum_r mean_r(h1_src) @ W2[r])
  out[g] = mean over nodes in graph g of h2            -> [64, 128] f32

Distribution: nodes (and their incoming edges) are sharded over 8 cores by
destination id (12544 nodes/core). Per-relation weights are replicated.
Layer-1 edge features (15-dim x rows, scaled by 1/deg) are pre-gathered on
the host as part of input sharding. Layer-2 features (h1, device-computed)
are exchanged with a chunked AllGather (overlapped under layer-1 compute)
and gathered on-device via the batched gpsimd dma_gather custom op (int16
indices -> the h1 table is split into 4 quarters; slot groups are packed
per (dst-window, src-quarter) so each gather instruction covers one
quarter's groups of one output tile, on its own SWDGE queue).
Aggregation uses a one-hot matmul: for each 128-edge group, a bf16
selection matrix built on the vector engine (iota + is_equal against
relation-folded window keys) scatters gathered rows into per-window PSUM
accumulators on the tensor engine, which also handles duplicate
destinations for free.
"""
import sys
sys.path.insert(0, "/opt/trn_rl_repo")
import numpy as np

import concourse.bass as bass
import concourse.mybir as mybir
import concourse.tile as tile_mod
from concourse.tile import TileContext
from concourse.bacc import Bacc
from concourse.ap import AP
from concourse.masks import make_identity
from concourse.tile_rust import add_dep_helper

# ---------------------------------------------------------------- constants
NCORES = 8
N = 100000
NPAD = 100352            # 8 * 12544
PC = NPAD // NCORES      # 12544 nodes per core
W = 16                   # dst window width (COLS = 4W one-hot columns)
NW = PC // W             # windows per core
COLS = 4 * W             # one-hot columns per window
NWPB = 512 // COLS       # windows per PSUM bank
TW = 256 // W            # 8 windows per output tile (256 nodes)
NT = NW // TW            # 49 output tiles
H = 128                  # hidden dim
F1 = 16                  # padded layer-1 input dim (15 real)
R = 4                    # relations
NGRAPH = 64
NQ = 4                   # src table quarters (dma_gather int16 index range)
QS = NPAD // NQ          # 25088 rows per quarter
NCHUNK = 7               # AllGather chunks
CH = PC // NCHUNK        # 1792 rows per chunk
CHT = NT // NCHUNK       # 7 tiles per chunk
CHUNK_AG = True          # chunked AllGather (overlap with L1) vs single
NQUEUES = 4              # SWDGE queues used for dma_gather (1..4)
GBMAX = 8                # max groups (1024 idxs) per dma_gather: the SWDGE
                         # descriptor ring holds 128 descs/engine; a single
                         # DMA needing >127 descs/engine is illegal

_bf16 = mybir.dt.bfloat16
_f32 = mybir.dt.float32
_i16 = mybir.dt.int16


def _to_bf16(a):
    """f32 -> bf16 (round-to-nearest-even) stored as numpy uint16 view array."""
    import ml_dtypes
    return a.astype(ml_dtypes.bfloat16)


# ------------------------------------------------------- tile/walrus patches
def _patch_tile_drain():
    """This deployment's walrus accepts only ONE sync-wait per instruction:
    split the end-of-TileContext drain into single-wait drains."""
    def _patched(self, tick_clock, wait_clock):
        nc = self.nc
        drain_inst = nc.sync.drain()
        wait_clock.add_sem_waits(
            drain_inst.ins, tile_mod.ScopedClock({None: tick_clock.global_clock})
        )
        si = drain_inst.ins.sync_info
        if si is not None and si.on_wait and len(si.on_wait) > 1:
            waits = list(si.on_wait)
            si.on_wait = waits[:1]
            for i in range(1, len(waits)):
                extra = nc.sync.drain()
                esi = extra.ins.sync_info
                if esi is None:
                    extra.ins.sync_info = mybir.SyncInfo(
                        on_wait=[waits[i]], on_update=[])
                else:
                    esi.on_wait = [waits[i]]
        nc.all_engine_barrier()
        assert self.sems is not None
        popped = nc._tile_sem_poison_stack.pop()
        assert popped is self._sem_poison
        nc.clear_and_free_semaphores(list(self.sems.allocated().values()))
        nc.all_engine_barrier()
    TileContext._drain_and_barrier = _patched


_patch_tile_drain()
_legal_ctr = [0]


def _legalize_waits(nc, maxw=1):
    """Split >maxw sync-waits on any instruction onto preceding same-engine
    NoOps (engine streams are in-order, so this is semantics-preserving)."""
    for f in nc.m.functions:
        for blk in f.blocks:
            insts = list(blk.instructions)
            out = []
            changed = False
            for ins in insts:
                si = ins.sync_info
                if si is not None and si.on_wait and len(si.on_wait) > maxw:
                    waits = list(si.on_wait)
                    for i in range(0, len(waits) - maxw, maxw):
                        _legal_ctr[0] += 1
                        nop = mybir.InstNoOp(
                            name=f"legalw-{_legal_ctr[0]}", ins=[], outs=[])
                        nop.engine = ins.engine
                        nop.sync_info = mybir.SyncInfo(
                            on_wait=waits[i:i + maxw], on_update=[])
                        out.append(nop)
                    si.on_wait = waits[len(waits) - maxw:]
                    changed = True
                out.append(ins)
            if changed:
                blk.instructions = out


# ------------------------------------------------------------- group layout
def _group_layout(cap_wq):
    """Group ordering: tile t -> quarter q -> window w -> j.
    Returns (seqs, tq, tile_g0, wq_goff, TOTAL_G):
      seqs[t]   = [(local_g, w_in_tile, q), ...]
      tq[t]     = [(q, local_g0, ngroups), ...]   gather calls for the tile
      tile_g0   = [NT+1] global group offset per tile
      wq_goff   = [NW, NQ] global group offset of cell (w, q)
    """
    seqs, tq = [], []
    tile_g0 = np.zeros(NT + 1, dtype=np.int64)
    wq_goff = np.zeros((NW, NQ), dtype=np.int64)
    g = 0
    for t in range(NT):
        tile_g0[t] = g
        seq, tqr = [], []
        for q in range(NQ):
            lg0 = g - tile_g0[t]
            for wi in range(TW):
                w = t * TW + wi
                wq_goff[w, q] = g
                for _ in range(int(cap_wq[w, q])):
                    seq.append((int(g - tile_g0[t]), wi, q))
                    g += 1
            ng = (g - tile_g0[t]) - lg0
            if ng:
                tqr.append((q, int(lg0), int(ng)))
        # matmuls must run window-major: a PSUM bank can only have ONE open
        # accumulation group at a time (the slot/gather layout stays
        # quarter-major; only the emission order changes)
        seq.sort(key=lambda e: e[1])
        seqs.append(seq)
        tq.append(tqr)
    tile_g0[NT] = g
    return seqs, tq, tile_g0, wq_goff, int(g)


def _table_row(node):
    """Global node id -> h1 table row (chunk-major when CHUNK_AG)."""
    if not CHUNK_AG:
        return node
    k = node // PC
    r = node % PC
    c = r // CH
    rr = r % CH
    return c * (NCORES * CH) + k * CH + rr


# ------------------------------------------------------------- host prep
def _host_prep(x, W1, root1, b1, W2, root2, b2, edge_index, edge_type, batch):
    """Shard/repack all inputs. Returns (per_core_inmaps, host_ctx)."""
    src = np.asarray(edge_index[0], dtype=np.int64)
    dst = np.asarray(edge_index[1], dtype=np.int64)
    rel = np.asarray(edge_type, dtype=np.int64)
    batch = np.asarray(batch, dtype=np.int64)
    x = np.asarray(x, dtype=np.float32)
    E = src.shape[0]

    # per-(relation, dst) in-degree counts -> mean scale
    cnt = np.zeros((R, N), dtype=np.int64)
    np.add.at(cnt, (rel, dst), 1)
    recip = (1.0 / np.maximum(cnt, 1)).astype(np.float32)   # [R, N]

    core_of = dst // PC
    woff = dst % PC
    win = woff // W
    key = rel * W + (woff % W)                               # [0, 4W)
    trow = _table_row(src)                                   # h1 table row
    quar = trow // QS
    qrel = (trow - quar * QS).astype(np.int16)               # [0, QS)

    # per-(core, window, quarter) counts -> shared capacities
    cwq = np.zeros((NCORES, NW, NQ), dtype=np.int64)
    np.add.at(cwq, (core_of, win, quar), 1)
    cap_wq = np.ceil(cwq.max(axis=0) / 128).astype(np.int64)  # [NW, NQ]

    seqs, tq, tile_g0, wq_goff, TOTAL_G = _group_layout(cap_wq)
    NSLOT = TOTAL_G * 128

    # slot assignment: sort edges by (core, window, quarter); edges of a
    # cell fill slots wq_goff[w,q]*128 ... in order
    order = np.lexsort((quar, win, core_of))
    s_src, s_rel, s_dst = src[order], rel[order], dst[order]
    s_core, s_win, s_quar = core_of[order], win[order], quar[order]
    s_key = key[order]
    s_qrel = qrel[order]
    s_scale = recip[s_rel, s_dst].astype(np.float32)

    cell_id = (s_core * NW + s_win) * NQ + s_quar
    cell_start = np.zeros(NCORES * NW * NQ + 1, dtype=np.int64)
    np.add.at(cell_start, cell_id + 1, 1)
    cell_start = np.cumsum(cell_start)
    pos_in_cell = np.arange(E) - cell_start[cell_id]
    slot = (wq_goff[s_win, s_quar] * 128 + pos_in_cell).astype(np.int64)

    keys_all = np.full((NCORES, NSLOT), -1.0, dtype=np.float32)
    idx_all = np.zeros((NCORES, NSLOT), dtype=np.int16)      # pad -> row 0
    xsl_all = np.zeros((NCORES, NSLOT, F1), dtype=np.float32)
    keys_all[s_core, slot] = s_key
    idx_all[s_core, slot] = s_qrel
    xsl_all[s_core, slot, :15] = x[s_src] * s_scale[:, None]

    # [core, 128, TOTAL_G] layouts (slot = g*128 + p  ->  [p, g])
    keys_pg = _to_bf16(
        keys_all.reshape(NCORES, TOTAL_G, 128).transpose(0, 2, 1).copy())
    xsl_pg = _to_bf16(
        xsl_all.reshape(NCORES, TOTAL_G, 128, F1).transpose(0, 2, 1, 3).copy())
    # int16 idx, 16-partition wrap replicated to 128: [p, j] = idx[j*16+p%16]
    idx16 = idx_all.reshape(NCORES, TOTAL_G * 8, 16).transpose(0, 2, 1)
    idx16 = np.tile(idx16, (1, 8, 1)).copy()                 # [core, 128, G*8]

    # scale columns for layer 2 (partition-replicated): col (w, r*W+o)
    sc = np.zeros((NCORES, NW * 4 * W), dtype=np.float32)
    for k in range(NCORES):
        r_grid = np.repeat(np.arange(R), W)  # [4W] -> r
        o_grid = np.tile(np.arange(W), R)    # [4W] -> offset
        nodes_w = (k * PC + (np.arange(NW)[:, None] * W + o_grid[None, :]))
        nodes_w = np.minimum(nodes_w, N - 1)
        sc[k] = recip[r_grid[None, :], nodes_w].reshape(-1)
    sc_bf = _to_bf16(sc)                     # [NCORES, NW*128]
    sc_rep = np.broadcast_to(
        sc_bf[:, None, :], (NCORES, 128, NW * 4 * W)).copy()

    # batch one-hot [PC, 64] per core
    bone = np.zeros((NCORES, PC, NGRAPH), dtype=np.float32)
    for k in range(NCORES):
        nd = k * PC + np.arange(PC)
        real = nd < N
        bone[k, real, batch[nd[real]]] = 1.0
    bone_bf = _to_bf16(bone)

    # x^T own block, padded feat rows [128, PC]
    xT = np.zeros((NCORES, 128, PC), dtype=np.float32)
    for k in range(NCORES):
        nd = k * PC + np.arange(PC)
        real = nd < N
        xT[k][:15][:, real] = x[nd[real]].T
    xT_bf = _to_bf16(xT)

    # weights (replicated)
    def padw(w, rows):
        out = np.zeros((128, H), dtype=np.float32)
        out[:rows] = w
        return _to_bf16(out)

    W1p = np.stack([padw(np.asarray(W1)[r], 15) for r in range(R)])
    root1p = padw(np.asarray(root1), 15)
    W2p = np.stack([padw(np.asarray(W2)[r], H) for r in range(R)])
    root2p = padw(np.asarray(root2), H)
    b1f = np.asarray(b1, dtype=np.float32).reshape(H, 1)
    b2f = np.asarray(b2, dtype=np.float32).reshape(H, 1)

    in_maps = []
    for k in range(NCORES):
        in_maps.append({
            "xsl": xsl_pg[k],          # [128, TOTAL_G, F1] bf16
            "keys": keys_pg[k],        # [128, TOTAL_G] bf16
            "idx16": idx16[k],         # [128, TOTAL_G*8] int16
            "screp": sc_rep[k],        # [128, NW*128] bf16
            "bone": bone_bf[k],        # [PC, 64] bf16
            "xT": xT_bf[k],            # [128, PC] bf16
            "W1p": W1p, "root1p": root1p,
            "W2p": W2p, "root2p": root2p,
            "b1": b1f, "b2": b2f,
        })

    gcounts = np.maximum(np.bincount(batch, minlength=NGRAPH), 1).astype(np.float32)
    host_ctx = {"cap_wq": cap_wq, "TOTAL_G": TOTAL_G, "gcounts": gcounts}
    return in_maps, host_ctx


# ------------------------------------------------------------- device build
def _build_nc(cap_wq, legalize=None):
    import os as _os
    if legalize is None:
        legalize = not _os.environ.get("BASS_NO_LEGALIZE")
    seqs, tq, tile_g0, wq_goff, TOTAL_G = _group_layout(cap_wq)
    GTMAX = int(max(tile_g0[t + 1] - tile_g0[t] for t in range(NT)))

    nc = Bacc("TRN2", num_devices=NCORES, num_swdge_queues=NQUEUES,
              dynamic_dma_scratch_size=32768)
    xsl = nc.dram_tensor("xsl", [128, TOTAL_G, F1], _bf16, kind="ExternalInput")
    keys = nc.dram_tensor("keys", [128, TOTAL_G], _bf16, kind="ExternalInput")
    idx16d = nc.dram_tensor("idx16", [128, TOTAL_G * 8], _i16,
                            kind="ExternalInput")
    screp = nc.dram_tensor("screp", [128, NW * COLS], _bf16, kind="ExternalInput")
    bone = nc.dram_tensor("bone", [PC, NGRAPH], _bf16, kind="ExternalInput")
    xT = nc.dram_tensor("xT", [128, PC], _bf16, kind="ExternalInput")
    W1p = nc.dram_tensor("W1p", [R, 128, H], _bf16, kind="ExternalInput")
    root1p = nc.dram_tensor("root1p", [128, H], _bf16, kind="ExternalInput")
    W2p = nc.dram_tensor("W2p", [R, 128, H], _bf16, kind="ExternalInput")
    root2p = nc.dram_tensor("root2p", [128, H], _bf16, kind="ExternalInput")
    b1 = nc.dram_tensor("b1", [H, 1], _f32, kind="ExternalInput")
    b2 = nc.dram_tensor("b2", [H, 1], _f32, kind="ExternalInput")
    h1own = nc.dram_tensor("h1own", [PC, H], _bf16, kind="Internal")
    h1tab = nc.dram_tensor("h1tab", [NPAD, H], _bf16, kind="Internal",
                           addr_space="Shared")
    pool_out = nc.dram_tensor("pool_out", [NGRAPH, H], _f32, kind="ExternalOutput")

    with TileContext(nc, num_cores=NCORES) as tc:
        import contextlib
        with contextlib.ExitStack() as ctx:
            const_p = ctx.enter_context(tc.tile_pool(name="const", bufs=1))
            wpool = ctx.enter_context(tc.tile_pool(name="wts", bufs=1))
            hpool = ctx.enter_context(tc.tile_pool(name="hT", bufs=1))
            feed_p = ctx.enter_context(tc.tile_pool(name="feed", bufs=3))
            oh_p = ctx.enter_context(tc.tile_pool(name="oh", bufs=3))
            g_p = ctx.enter_context(tc.tile_pool(name="gat", bufs=3))
            sb_p = ctx.enter_context(tc.tile_pool(name="stile", bufs=2))
            sc_p = ctx.enter_context(tc.tile_pool(name="sctile", bufs=2))
            off_p = ctx.enter_context(tc.tile_pool(name="offt", bufs=3))
            tok_p = ctx.enter_context(tc.tile_pool(name="tok", bufs=3))
            bo_p = ctx.enter_context(tc.tile_pool(name="bo", bufs=3))
            misc_p = ctx.enter_context(tc.tile_pool(name="misc", bufs=2))
            ps_agg = ctx.enter_context(
                tc.tile_pool(name="ps_agg", bufs=1, space="PSUM"))
            ps_out = ctx.enter_context(
                tc.tile_pool(name="ps_out", bufs=1, space="PSUM"))
            ps_tr = ctx.enter_context(
                tc.tile_pool(name="ps_tr", bufs=2, space="PSUM"))
            ps_pool = ctx.enter_context(
                tc.tile_pool(name="ps_pool", bufs=1, space="PSUM"))

            # constants
            iota_i = const_p.tile([128, COLS], mybir.dt.int32)
            nc.gpsimd.iota(iota_i[:], pattern=[[1, COLS]],
                           base=0, channel_multiplier=0)
            iota_bf = const_p.tile([128, COLS], _bf16)
            nc.vector.tensor_copy(iota_bf[:], iota_i[:])
            ident = const_p.tile([128, 128], _bf16)
            make_identity(nc, ident[:])

            # weights resident in SBUF
            w1t = [wpool.tile([128, H], _bf16, tag=f"w1_{r}", name=f"w1_{r}")
                   for r in range(R)]
            w2t = [wpool.tile([128, H], _bf16, tag=f"w2_{r}", name=f"w2_{r}")
                   for r in range(R)]
            r1t = wpool.tile([128, H], _bf16, tag="r1")
            r2t = wpool.tile([128, H], _bf16, tag="r2")
            b1t = wpool.tile([H, 1], _f32, tag="b1")
            b2t = wpool.tile([H, 1], _f32, tag="b2")
            for r in range(R):
                nc.sync.dma_start(out=w1t[r][:], in_=W1p[r])
                nc.sync.dma_start(out=w2t[r][:], in_=W2p[r])
            nc.sync.dma_start(out=r1t[:], in_=root1p[:, :])
            nc.sync.dma_start(out=r2t[:], in_=root2p[:, :])
            nc.sync.dma_start(out=b1t[:], in_=b1[:, :])
            nc.sync.dma_start(out=b2t[:], in_=b2[:, :])

            hT_x = hpool.tile([128, PC], _bf16, tag="hT_x")     # layer1 rhs
            hT_1 = hpool.tile([128, PC], _bf16, tag="hT_1")     # layer1 out
            nc.sync.dma_start(out=hT_x[:], in_=xT[:, :])

            pool_acc = const_p.tile([NGRAPH, H], _f32)
            nc.vector.memset(pool_acc[:], 0.0)

            def bcast_inner(tile_ap, ncols, inner):
                """[128, ncols] -> AP [128, ncols, inner] (step-0 inner)."""
                base = tile_ap
                newap = [list(base.ap[0]), [base.ap[-1][0], ncols], [0, inner]]
                return AP(base.tensor, base.offset, newap)

            def bcast_mid(tile_ap, nmid):
                """[128, COLS] -> AP [128, nmid, COLS] (step-0 middle)."""
                base = tile_ap
                newap = [list(base.ap[0]), [0, nmid], [base.ap[-1][0], COLS]]
                return AP(base.tensor, base.offset, newap)

            ag_insts = []

            # per-window group totals per tile (for start/stop flags)
            def layer(L, hT_in, wts, roott, bt, hT_out, ag_deps=None):
                dep_done = [False]
                for t in range(NT):
                    g0, g1 = int(tile_g0[t]), int(tile_g0[t + 1])
                    GT = g1 - g0
                    if L == 2:
                        idxt = off_p.tile([128, GTMAX * 8], _i16, tag="idxt")
                        nc.sync.dma_start(out=idxt[:, :GT * 8],
                                          in_=idx16d[:, g0 * 8:g1 * 8])
                        gt_all = g_p.tile([128, GTMAX * H], _bf16, tag="gt")
                        for (q, qg0, qng) in tq[t]:
                            for lg0 in range(qg0, qg0 + qng, GBMAX):
                                ng = min(GBMAX, qg0 + qng - lg0)
                                gi = nc.gpsimd.dma_gather(
                                    gt_all[:, lg0 * H:
                                           (lg0 + ng) * H].rearrange(
                                               "p (g h) -> p g h", h=H),
                                    h1tab[q * QS:(q + 1) * QS, :],
                                    idxt[:, lg0 * 8:(lg0 + ng) * 8],
                                    num_idxs=ng * 128,
                                    num_idxs_reg=ng * 128,
                                    elem_size=H,
                                    elem_step=H,
                                    single_packet=False,
                                    queue_num=q % NQUEUES)
                                if ag_deps and not dep_done[0]:
                                    for ad in ag_deps:
                                        add_dep_helper(
                                            gi.ins, ad,
                                            reason="L2 gather after AG")
                                    dep_done[0] = True
                    else:
                        feedt = feed_p.tile([128, GTMAX, F1], _bf16,
                                            tag="feed")
                        nc.sync.dma_start(
                            out=feedt[:, :GT, :],
                            in_=xsl[:, g0:g1, :])
                    keyt = misc_p.tile([128, GTMAX], _bf16, tag="keyt")
                    nc.sync.dma_start(out=keyt[:, :GT], in_=keys[:, g0:g1])
                    # one-hot for all groups of this tile in one DVE op
                    oht = oh_p.tile([128, GTMAX * COLS], _bf16, tag="oht")
                    nc.vector.tensor_tensor(
                        out=oht[:, :GT * COLS],
                        in0=bcast_mid(iota_bf[:], GT),
                        in1=bcast_inner(keyt[:, :GT], GT, COLS),
                        op=mybir.AluOpType.is_equal)

                    # aggregation PSUM: two banks of 4 windows
                    agg = [ps_agg.tile([128, 512], _f32, tag=f"agg{h}",
                                       name=f"agg{h}") for h in range(2)]
                    totals = [int(cap_wq[t * TW + wi, :].sum())
                              for wi in range(TW)]
                    emitted = [0] * TW
                    M = H if L == 2 else F1
                    for (lg, wi, q) in seqs[t]:
                        ps = agg[wi // NWPB]
                        colsl = slice((wi % NWPB) * COLS,
                                      (wi % NWPB) * COLS + COLS)
                        if L == 2:
                            lhs = gt_all[:, lg * H:(lg + 1) * H]
                        else:
                            lhs = feedt[:, lg, :]
                        nc.tensor.matmul(
                            ps[:M, colsl],
                            lhsT=lhs,
                            rhs=oht[:, lg * COLS:(lg + 1) * COLS],
                            start=(emitted[wi] == 0),
                            stop=(emitted[wi] == totals[wi] - 1))
                        emitted[wi] += 1

                    # scale (L2) / plain (L1) copy PSUM -> SBUF S~^T bf16
                    KF = H if L == 2 else F1
                    st = sb_p.tile([128, TW * COLS], _bf16, tag="st")
                    if L == 2:
                        sct = sc_p.tile([128, TW * COLS], _bf16, tag="sct")
                        nc.sync.dma_start(
                            out=sct[:], in_=screp[:, t * TW * COLS:
                                                  (t + 1) * TW * COLS])
                        for h in range(2):
                            nc.vector.tensor_tensor(
                                out=st[:, h * 512:(h + 1) * 512],
                                in0=agg[h][:, :],
                                in1=sct[:, h * 512:(h + 1) * 512],
                                op=mybir.AluOpType.mult)
                    else:
                        for h in range(2):
                            nc.vector.tensor_copy(
                                st[:KF, h * 512:(h + 1) * 512],
                                agg[h][:KF, :])

                    # transform: out2^T [128, 256 nodes]
                    op_ps = ps_out.tile([128, 256], _f32, tag="ops")
                    nsl = slice(t * 256, (t + 1) * 256)
                    nc.tensor.matmul(op_ps[:, :], lhsT=roott[:KF, :],
                                     rhs=hT_in[:KF, nsl], start=True, stop=False)
                    st3 = st[:KF, :].rearrange("p (a b) -> p a b", b=COLS)
                    for r in range(R):
                        nc.tensor.matmul(op_ps[:, :], lhsT=wts[r][:KF, :],
                                         rhs=st3[:, :, r * W:(r + 1) * W],
                                         start=False,
                                         stop=(r == R - 1))
                    # bias + relu -> hT_out (bf16)
                    nc.scalar.activation(
                        out=hT_out[:, nsl], in_=op_ps[:, :],
                        func=mybir.ActivationFunctionType.Relu,
                        bias=bt[:], scale=1.0)

                    # transpose to token-major for h1 table / pooling
                    for half in range(2):
                        tr = ps_tr.tile([128, 128], _bf16, tag="tr")
                        nc.tensor.transpose(
                            tr[:, :],
                            hT_out[:, t * 256 + half * 128:
                                   t * 256 + (half + 1) * 128],
                            ident[:])
                        tok = tok_p.tile([128, 128], _bf16, tag="tok")
                        nc.scalar.activation(
                            out=tok[:], in_=tr[:, :],
                            func=mybir.ActivationFunctionType.Copy)
                        row0 = t * 256 + half * 128
                        if L == 1:
                            nc.sync.dma_start(
                                out=h1own[row0:row0 + 128, :], in_=tok[:])
                        else:
                            bt_t = bo_p.tile([128, NGRAPH], _bf16, tag="bt")
                            nc.sync.dma_start(
                                out=bt_t[:], in_=bone[row0:row0 + 128, :])
                            pp = ps_pool.tile([NGRAPH, H], _f32, tag="pp")
                            nc.tensor.matmul(pp[:, :], lhsT=bt_t[:],
                                             rhs=tok[:], start=True, stop=True)
                            nc.vector.tensor_tensor(
                                out=pool_acc[:], in0=pool_acc[:], in1=pp[:, :],
                                op=mybir.AluOpType.add)

                    # chunked AllGather: publish h1 rows as soon as a chunk
                    # of 7 tiles is complete (overlaps remaining L1 compute)
                    if CHUNK_AG and L == 1 and (t % CHT) == CHT - 1:
                        c = t // CHT
                        ag = nc.gpsimd.collective_compute(
                            "AllGather", mybir.AluOpType.bypass,
                            replica_groups=[list(range(NCORES))],
                            ins=[h1own[c * CH:(c + 1) * CH, :]],
                            outs=[h1tab[c * NCORES * CH:
                                        (c + 1) * NCORES * CH, :]])
                        ag_insts.append(ag.ins if hasattr(ag, "ins") else ag)

            layer(1, hT_x, w1t, r1t, b1t, hT_1)
            if not CHUNK_AG:
                ag = nc.gpsimd.collective_compute(
                    "AllGather", mybir.AluOpType.bypass,
                    replica_groups=[list(range(NCORES))],
                    ins=[h1own[:, :]], outs=[h1tab[:, :]])
                ag_insts.append(ag.ins if hasattr(ag, "ins") else ag)
            # hT_x is fully consumed by layer 1 -> reuse its SBUF for h2^T
            layer(2, hT_1, w2t, r2t, b2t, hT_x, ag_deps=ag_insts)

            nc.sync.dma_start(out=pool_out[:, :], in_=pool_acc[:])

    nc.finalize()
    if legalize:
        _legalize_waits(nc)
    return nc


# ------------------------------------------------------------- runner
_CACHE = {}


def _get_compiled(cap_wq):
    key = ("nc", tuple(cap_wq.reshape(-1).tolist()))
    if key not in _CACHE:
        import jax
        from jax.sharding import Mesh, PartitionSpec
        from jax.experimental.shard_map import shard_map
        from concourse.bass2jax import (
            _bass_exec_p, partition_id_tensor, install_neuronx_cc_hook)
        install_neuronx_cc_hook()
        nc = _build_nc(cap_wq)

        partition_name = (nc.partition_id_tensor.name
                          if nc.partition_id_tensor else None)
        in_names, out_names, out_avals = [], [], []
        for alloc in nc.m.functions[0].allocations:
            if not isinstance(alloc, mybir.MemoryLocationSet):
                continue
            name = alloc.memorylocations[0].name
            if alloc.kind == "ExternalInput":
                if name != partition_name and name != (
                        nc.dbg_addr.name if nc.dbg_addr is not None else None):
                    in_names.append(name)
            elif alloc.kind == "ExternalOutput":
                out_names.append(name)
                out_avals.append(jax.core.ShapedArray(
                    tuple(alloc.tensor_shape), mybir.dt.np(alloc.dtype)))
        n_params, n_outs = len(in_names), len(out_names)
        all_in = list(in_names) + list(out_names)
        if nc.dbg_addr is not None:
            all_in.append(nc.dbg_addr.name)
        if partition_name is not None:
            all_in.append(partition_name)

        def _body(*args):
            operands = list(args)
            if nc.dbg_addr is not None:
                operands.append(jax.numpy.zeros((1, 2), jax.numpy.uint32))
            if partition_name is not None:
                operands.append(partition_id_tensor())
            outs = _bass_exec_p.bind(
                *operands, out_avals=tuple(out_avals),
                in_names=tuple(all_in), out_names=tuple(out_names),
                lowering_input_output_aliases=(),
                sim_require_finite=False, sim_require_nnan=False, nc=nc)
            return tuple(outs)

        devices = jax.devices()[:NCORES]
        mesh = Mesh(np.asarray(devices), ("core",))
        import os as _os
        donate = (() if _os.environ.get("BASS_NO_DONATE")
                  else tuple(range(n_params, n_params + n_outs)))
        sharded = jax.jit(
            shard_map(_body, mesh=mesh,
                      in_specs=(PartitionSpec("core"),) * (n_params + n_outs),
                      out_specs=(PartitionSpec("core"),) * n_outs,
                      check_rep=False),
            donate_argnums=donate,
            keep_unused=True)
        _CACHE[key] = (sharded, in_names, out_names, out_avals, mesh)
    return _CACHE[key]


def run_device(in_maps, cap_wq):
    import jax
    sharded, in_names, out_names, out_avals, mesh = _get_compiled(cap_wq)
    concat_in = [
        np.concatenate([np.asarray(in_maps[c][name]) for c in range(NCORES)],
                       axis=0)
        for name in in_names]
    concat_zeros = [
        np.zeros((NCORES * a.shape[0], *a.shape[1:]), a.dtype)
        for a in out_avals]
    out_arrs = sharded(*concat_in, *concat_zeros)
    jax.block_until_ready(out_arrs)
    res = [
        {name: np.asarray(out_arrs[i]).reshape(NCORES, *out_avals[i].shape)[c]
         for i, name in enumerate(out_names)}
        for c in range(NCORES)]
    return res


def kernel(x, W1, root1, b1, W2, root2, b2, edge_index, edge_type, batch):
    in_maps, hc = _host_prep(x, W1, root1, b1, W2, root2, b2,
                             edge_index, edge_type, batch)
    res = run_device(in_maps, hc["cap_wq"])
    total = np.zeros((NGRAPH, H), dtype=np.float32)
    for k in range(NCORES):
        total += res[k]["pool_out"]
    return (total / hc["gcounts"][:, None]).astype(np.float32)


# revision 21
# speedup vs baseline: 2.0969x; 1.0191x over previous
"""BasicRGCN Trainium2 kernel — 8-core SPMD Bass/Tile implementation.

Model (PyG-style RGCNConv x2 + global_mean_pool):
  h1 = relu(x @ root1 + b1 + sum_r mean_r(x_src) @ W1[r])
  h2 = relu(h1 @ root2 + b2 + sum_r mean_r(h1_src) @ W2[r])
  out[g] = mean over nodes in graph g of h2            -> [64, 128] f32

Distribution: nodes (and their incoming edges) are sharded over 8 cores by
destination id (12544 nodes/core). Per-relation weights are replicated.
Layer-1 edge features (15-dim x rows, scaled by 1/deg) are pre-gathered on
the host as part of input sharding. Layer-2 features (h1, device-computed)
are exchanged with a chunked AllGather (overlapped under layer-1 compute)
and gathered on-device via the batched gpsimd dma_gather custom op (int16
indices -> the h1 table is split into 4 quarters; slot groups are packed
per (dst-window, src-quarter) so each gather instruction covers one
quarter's groups of one output tile, on its own SWDGE queue).
Aggregation uses a one-hot matmul: for each 128-edge group, a bf16
selection matrix built on the vector engine (iota + is_equal against
relation-folded window keys) scatters gathered rows into per-window PSUM
accumulators on the tensor engine, which also handles duplicate
destinations for free.
"""
import sys
sys.path.insert(0, "/opt/trn_rl_repo")
import numpy as np

import concourse.bass as bass
import concourse.mybir as mybir
import concourse.tile as tile_mod
from concourse.tile import TileContext
from concourse.bacc import Bacc
from concourse.ap import AP
from concourse.masks import make_identity
from concourse.tile_rust import add_dep_helper

# ---------------------------------------------------------------- constants
NCORES = 8
N = 100000
NPAD = 100352            # 8 * 12544
PC = NPAD // NCORES      # 12544 nodes per core
W = 16                   # dst window width (COLS = 4W one-hot columns)
NW = PC // W             # windows per core
COLS = 4 * W             # one-hot columns per window
NWPB = 512 // COLS       # windows per PSUM bank
TW = 256 // W            # 8 windows per output tile (256 nodes)
NT = NW // TW            # 49 output tiles
H = 128                  # hidden dim
F1 = 16                  # padded layer-1 input dim (15 real)
R = 4                    # relations
NGRAPH = 64
NQ = 4                   # src table quarters (dma_gather int16 index range)
QS = NPAD // NQ          # 25088 rows per quarter
NCHUNK = 7               # AllGather chunks
CH = PC // NCHUNK        # 1792 rows per chunk
CHT = NT // NCHUNK       # 7 tiles per chunk
CHUNK_AG = True          # chunked AllGather (overlap with L1) vs single
NQUEUES = 4              # SWDGE queues used for dma_gather (1..4)
GBMAX = 16               # max groups per dma_gather, descriptor-ring bound:
                         # ring holds dynamic_dma_scratch_size/128 descs per
                         # engine; a DMA needing more is illegal

_bf16 = mybir.dt.bfloat16
_f32 = mybir.dt.float32
_i16 = mybir.dt.int16


def _to_bf16(a):
    """f32 -> bf16 (round-to-nearest-even) stored as numpy uint16 view array."""
    import ml_dtypes
    return a.astype(ml_dtypes.bfloat16)


# ------------------------------------------------------- tile/walrus patches
def _patch_tile_drain():
    """This deployment's walrus accepts only ONE sync-wait per instruction:
    split the end-of-TileContext drain into single-wait drains."""
    def _patched(self, tick_clock, wait_clock):
        nc = self.nc
        drain_inst = nc.sync.drain()
        wait_clock.add_sem_waits(
            drain_inst.ins, tile_mod.ScopedClock({None: tick_clock.global_clock})
        )
        si = drain_inst.ins.sync_info
        if si is not None and si.on_wait and len(si.on_wait) > 1:
            waits = list(si.on_wait)
            si.on_wait = waits[:1]
            for i in range(1, len(waits)):
                extra = nc.sync.drain()
                esi = extra.ins.sync_info
                if esi is None:
                    extra.ins.sync_info = mybir.SyncInfo(
                        on_wait=[waits[i]], on_update=[])
                else:
                    esi.on_wait = [waits[i]]
        nc.all_engine_barrier()
        assert self.sems is not None
        popped = nc._tile_sem_poison_stack.pop()
        assert popped is self._sem_poison
        nc.clear_and_free_semaphores(list(self.sems.allocated().values()))
        nc.all_engine_barrier()
    TileContext._drain_and_barrier = _patched


_patch_tile_drain()
_legal_ctr = [0]


def _legalize_waits(nc, maxw=1):
    """Split >maxw sync-waits on any instruction onto preceding same-engine
    NoOps (engine streams are in-order, so this is semantics-preserving)."""
    for f in nc.m.functions:
        for blk in f.blocks:
            insts = list(blk.instructions)
            out = []
            changed = False
            for ins in insts:
                si = ins.sync_info
                if si is not None and si.on_wait and len(si.on_wait) > maxw:
                    waits = list(si.on_wait)
                    for i in range(0, len(waits) - maxw, maxw):
                        _legal_ctr[0] += 1
                        nop = mybir.InstNoOp(
                            name=f"legalw-{_legal_ctr[0]}", ins=[], outs=[])
                        nop.engine = ins.engine
                        nop.sync_info = mybir.SyncInfo(
                            on_wait=waits[i:i + maxw], on_update=[])
                        out.append(nop)
                    si.on_wait = waits[len(waits) - maxw:]
                    changed = True
                out.append(ins)
            if changed:
                blk.instructions = out


# ------------------------------------------------------------- group layout
def _group_layout(cap_wq):
    """Group ordering: tile t -> quarter q -> window w -> j.
    Returns (seqs, tq, tile_g0, wq_goff, TOTAL_G):
      seqs[t]   = [(local_g, w_in_tile, q), ...]
      tq[t]     = [(q, local_g0, ngroups), ...]   gather calls for the tile
      tile_g0   = [NT+1] global group offset per tile
      wq_goff   = [NW, NQ] global group offset of cell (w, q)
    """
    seqs, tq = [], []
    tile_g0 = np.zeros(NT + 1, dtype=np.int64)
    wq_goff = np.zeros((NW, NQ), dtype=np.int64)
    g = 0
    for t in range(NT):
        tile_g0[t] = g
        seq, tqr = [], []
        for q in range(NQ):
            lg0 = g - tile_g0[t]
            for wi in range(TW):
                w = t * TW + wi
                wq_goff[w, q] = g
                for _ in range(int(cap_wq[w, q])):
                    seq.append((int(g - tile_g0[t]), wi, q))
                    g += 1
            ng = (g - tile_g0[t]) - lg0
            if ng:
                tqr.append((q, int(lg0), int(ng)))
        # matmuls must run window-major: a PSUM bank can only have ONE open
        # accumulation group at a time (the slot/gather layout stays
        # quarter-major; only the emission order changes)
        seq.sort(key=lambda e: e[1])
        seqs.append(seq)
        tq.append(tqr)
    tile_g0[NT] = g
    return seqs, tq, tile_g0, wq_goff, int(g)


def _table_row(node):
    """Global node id -> h1 table row (chunk-major when CHUNK_AG)."""
    if not CHUNK_AG:
        return node
    k = node // PC
    r = node % PC
    c = r // CH
    rr = r % CH
    return c * (NCORES * CH) + k * CH + rr


# ------------------------------------------------------------- host prep
def _host_prep(x, W1, root1, b1, W2, root2, b2, edge_index, edge_type, batch):
    """Shard/repack all inputs. Returns (per_core_inmaps, host_ctx)."""
    src = np.asarray(edge_index[0], dtype=np.int64)
    dst = np.asarray(edge_index[1], dtype=np.int64)
    rel = np.asarray(edge_type, dtype=np.int64)
    batch = np.asarray(batch, dtype=np.int64)
    x = np.asarray(x, dtype=np.float32)
    E = src.shape[0]

    # per-(relation, dst) in-degree counts -> mean scale
    cnt = np.zeros((R, N), dtype=np.int64)
    np.add.at(cnt, (rel, dst), 1)
    recip = (1.0 / np.maximum(cnt, 1)).astype(np.float32)   # [R, N]

    core_of = dst // PC
    woff = dst % PC
    win = woff // W
    key = rel * W + (woff % W)                               # [0, 4W)
    trow = _table_row(src)                                   # h1 table row
    quar = trow // QS
    qrel = (trow - quar * QS).astype(np.int16)               # [0, QS)

    # per-(core, window, quarter) counts -> shared capacities
    cwq = np.zeros((NCORES, NW, NQ), dtype=np.int64)
    np.add.at(cwq, (core_of, win, quar), 1)
    cap_wq = np.ceil(cwq.max(axis=0) / 128).astype(np.int64)  # [NW, NQ]

    seqs, tq, tile_g0, wq_goff, TOTAL_G = _group_layout(cap_wq)
    NSLOT = TOTAL_G * 128

    # slot assignment: sort edges by (core, window, quarter); edges of a
    # cell fill slots wq_goff[w,q]*128 ... in order
    order = np.lexsort((qrel, quar, win, core_of))
    s_src, s_rel, s_dst = src[order], rel[order], dst[order]
    s_core, s_win, s_quar = core_of[order], win[order], quar[order]
    s_key = key[order]
    s_qrel = qrel[order]
    s_scale = recip[s_rel, s_dst].astype(np.float32)

    cell_id = (s_core * NW + s_win) * NQ + s_quar
    cell_start = np.zeros(NCORES * NW * NQ + 1, dtype=np.int64)
    np.add.at(cell_start, cell_id + 1, 1)
    cell_start = np.cumsum(cell_start)
    pos_in_cell = np.arange(E) - cell_start[cell_id]
    slot = (wq_goff[s_win, s_quar] * 128 + pos_in_cell).astype(np.int64)

    keys_all = np.full((NCORES, NSLOT), -1.0, dtype=np.float32)
    idx_all = np.zeros((NCORES, NSLOT), dtype=np.int16)      # pad -> row 0
    xsl_all = np.zeros((NCORES, NSLOT, F1), dtype=np.float32)
    keys_all[s_core, slot] = s_key
    idx_all[s_core, slot] = s_qrel
    xsl_all[s_core, slot, :15] = x[s_src] * s_scale[:, None]

    # [core, 128, TOTAL_G] layouts (slot = g*128 + p  ->  [p, g])
    keys_pg = _to_bf16(
        keys_all.reshape(NCORES, TOTAL_G, 128).transpose(0, 2, 1).copy())
    xsl_pg = _to_bf16(
        xsl_all.reshape(NCORES, TOTAL_G, 128, F1).transpose(0, 2, 1, 3).copy())
    # int16 idx, 16-partition wrap replicated to 128: [p, j] = idx[j*16+p%16]
    idx16 = idx_all.reshape(NCORES, TOTAL_G * 8, 16).transpose(0, 2, 1)
    idx16 = np.tile(idx16, (1, 8, 1)).copy()                 # [core, 128, G*8]

    # scale columns for layer 2 (partition-replicated): col (w, r*W+o)
    sc = np.zeros((NCORES, NW * 4 * W), dtype=np.float32)
    for k in range(NCORES):
        r_grid = np.repeat(np.arange(R), W)  # [4W] -> r
        o_grid = np.tile(np.arange(W), R)    # [4W] -> offset
        nodes_w = (k * PC + (np.arange(NW)[:, None] * W + o_grid[None, :]))
        nodes_w = np.minimum(nodes_w, N - 1)
        sc[k] = recip[r_grid[None, :], nodes_w].reshape(-1)
    sc_bf = _to_bf16(sc)                     # [NCORES, NW*128]
    sc_rep = np.broadcast_to(
        sc_bf[:, None, :], (NCORES, 128, NW * 4 * W)).copy()

    # batch one-hot [PC, 64] per core
    bone = np.zeros((NCORES, PC, NGRAPH), dtype=np.float32)
    for k in range(NCORES):
        nd = k * PC + np.arange(PC)
        real = nd < N
        bone[k, real, batch[nd[real]]] = 1.0
    bone_bf = _to_bf16(bone)

    # x^T own block, padded feat rows [128, PC]
    xT = np.zeros((NCORES, 128, PC), dtype=np.float32)
    for k in range(NCORES):
        nd = k * PC + np.arange(PC)
        real = nd < N
        xT[k][:15][:, real] = x[nd[real]].T
    xT_bf = _to_bf16(xT)

    # weights (replicated)
    def padw(w, rows):
        out = np.zeros((128, H), dtype=np.float32)
        out[:rows] = w
        return _to_bf16(out)

    W1p = np.stack([padw(np.asarray(W1)[r], 15) for r in range(R)])
    root1p = padw(np.asarray(root1), 15)
    W2p = np.stack([padw(np.asarray(W2)[r], H) for r in range(R)])
    root2p = padw(np.asarray(root2), H)
    b1f = np.asarray(b1, dtype=np.float32).reshape(H, 1)
    b2f = np.asarray(b2, dtype=np.float32).reshape(H, 1)

    in_maps = []
    for k in range(NCORES):
        in_maps.append({
            "xsl": xsl_pg[k],          # [128, TOTAL_G, F1] bf16
            "keys": keys_pg[k],        # [128, TOTAL_G] bf16
            "idx16": idx16[k],         # [128, TOTAL_G*8] int16
            "screp": sc_rep[k],        # [128, NW*128] bf16
            "bone": bone_bf[k],        # [PC, 64] bf16
            "xT": xT_bf[k],            # [128, PC] bf16
            "W1p": W1p, "root1p": root1p,
            "W2p": W2p, "root2p": root2p,
            "b1": b1f, "b2": b2f,
        })

    gcounts = np.maximum(np.bincount(batch, minlength=NGRAPH), 1).astype(np.float32)
    host_ctx = {"cap_wq": cap_wq, "TOTAL_G": TOTAL_G, "gcounts": gcounts}
    return in_maps, host_ctx


# ------------------------------------------------------------- device build
def _build_nc(cap_wq, legalize=None):
    import os as _os
    if legalize is None:
        legalize = not _os.environ.get("BASS_NO_LEGALIZE")
    seqs, tq, tile_g0, wq_goff, TOTAL_G = _group_layout(cap_wq)
    GTMAX = int(max(tile_g0[t + 1] - tile_g0[t] for t in range(NT)))

    nc = Bacc("TRN2", num_devices=NCORES, num_swdge_queues=NQUEUES,
              dynamic_dma_scratch_size=32768)
    xsl = nc.dram_tensor("xsl", [128, TOTAL_G, F1], _bf16, kind="ExternalInput")
    keys = nc.dram_tensor("keys", [128, TOTAL_G], _bf16, kind="ExternalInput")
    idx16d = nc.dram_tensor("idx16", [128, TOTAL_G * 8], _i16,
                            kind="ExternalInput")
    screp = nc.dram_tensor("screp", [128, NW * COLS], _bf16, kind="ExternalInput")
    bone = nc.dram_tensor("bone", [PC, NGRAPH], _bf16, kind="ExternalInput")
    xT = nc.dram_tensor("xT", [128, PC], _bf16, kind="ExternalInput")
    W1p = nc.dram_tensor("W1p", [R, 128, H], _bf16, kind="ExternalInput")
    root1p = nc.dram_tensor("root1p", [128, H], _bf16, kind="ExternalInput")
    W2p = nc.dram_tensor("W2p", [R, 128, H], _bf16, kind="ExternalInput")
    root2p = nc.dram_tensor("root2p", [128, H], _bf16, kind="ExternalInput")
    b1 = nc.dram_tensor("b1", [H, 1], _f32, kind="ExternalInput")
    b2 = nc.dram_tensor("b2", [H, 1], _f32, kind="ExternalInput")
    h1own = nc.dram_tensor("h1own", [PC, H], _bf16, kind="Internal")
    h1tab = nc.dram_tensor("h1tab", [NPAD, H], _bf16, kind="Internal",
                           addr_space="Shared")
    pool_out = nc.dram_tensor("pool_out", [NGRAPH, H], _f32, kind="ExternalOutput")

    with TileContext(nc, num_cores=NCORES) as tc:
        import contextlib
        with contextlib.ExitStack() as ctx:
            const_p = ctx.enter_context(tc.tile_pool(name="const", bufs=1))
            wpool = ctx.enter_context(tc.tile_pool(name="wts", bufs=1))
            hpool = ctx.enter_context(tc.tile_pool(name="hT", bufs=1))
            feed_p = ctx.enter_context(tc.tile_pool(name="feed", bufs=3))
            oh_p = ctx.enter_context(tc.tile_pool(name="oh", bufs=3))
            g_p = ctx.enter_context(tc.tile_pool(name="gat", bufs=3))
            sb_p = ctx.enter_context(tc.tile_pool(name="stile", bufs=2))
            sc_p = ctx.enter_context(tc.tile_pool(name="sctile", bufs=2))
            off_p = ctx.enter_context(tc.tile_pool(name="offt", bufs=3))
            tok_p = ctx.enter_context(tc.tile_pool(name="tok", bufs=3))
            bo_p = ctx.enter_context(tc.tile_pool(name="bo", bufs=3))
            misc_p = ctx.enter_context(tc.tile_pool(name="misc", bufs=2))
            ps_agg = ctx.enter_context(
                tc.tile_pool(name="ps_agg", bufs=1, space="PSUM"))
            ps_out = ctx.enter_context(
                tc.tile_pool(name="ps_out", bufs=1, space="PSUM"))
            ps_tr = ctx.enter_context(
                tc.tile_pool(name="ps_tr", bufs=2, space="PSUM"))
            ps_pool = ctx.enter_context(
                tc.tile_pool(name="ps_pool", bufs=1, space="PSUM"))

            # constants
            iota_i = const_p.tile([128, COLS], mybir.dt.int32)
            nc.gpsimd.iota(iota_i[:], pattern=[[1, COLS]],
                           base=0, channel_multiplier=0)
            iota_bf = const_p.tile([128, COLS], _bf16)
            nc.vector.tensor_copy(iota_bf[:], iota_i[:])
            ident = const_p.tile([128, 128], _bf16)
            make_identity(nc, ident[:])

            # weights resident in SBUF
            w1t = [wpool.tile([128, H], _bf16, tag=f"w1_{r}", name=f"w1_{r}")
                   for r in range(R)]
            w2t = [wpool.tile([128, H], _bf16, tag=f"w2_{r}", name=f"w2_{r}")
                   for r in range(R)]
            r1t = wpool.tile([128, H], _bf16, tag="r1")
            r2t = wpool.tile([128, H], _bf16, tag="r2")
            b1t = wpool.tile([H, 1], _f32, tag="b1")
            b2t = wpool.tile([H, 1], _f32, tag="b2")
            for r in range(R):
                nc.sync.dma_start(out=w1t[r][:], in_=W1p[r])
                nc.sync.dma_start(out=w2t[r][:], in_=W2p[r])
            nc.sync.dma_start(out=r1t[:], in_=root1p[:, :])
            nc.sync.dma_start(out=r2t[:], in_=root2p[:, :])
            nc.sync.dma_start(out=b1t[:], in_=b1[:, :])
            nc.sync.dma_start(out=b2t[:], in_=b2[:, :])

            hT_x = hpool.tile([128, PC], _bf16, tag="hT_x")     # layer1 rhs
            hT_1 = hpool.tile([128, PC], _bf16, tag="hT_1")     # layer1 out
            nc.sync.dma_start(out=hT_x[:], in_=xT[:, :])

            pool_acc = const_p.tile([NGRAPH, H], _f32)
            nc.vector.memset(pool_acc[:], 0.0)

            def bcast_inner(tile_ap, ncols, inner):
                """[128, ncols] -> AP [128, ncols, inner] (step-0 inner)."""
                base = tile_ap
                newap = [list(base.ap[0]), [base.ap[-1][0], ncols], [0, inner]]
                return AP(base.tensor, base.offset, newap)

            def bcast_mid(tile_ap, nmid):
                """[128, COLS] -> AP [128, nmid, COLS] (step-0 middle)."""
                base = tile_ap
                newap = [list(base.ap[0]), [0, nmid], [base.ap[-1][0], COLS]]
                return AP(base.tensor, base.offset, newap)

            ag_insts = []

            # per-window group totals per tile (for start/stop flags)
            def layer(L, hT_in, wts, roott, bt, hT_out, ag_deps=None):
                dep_done = [False]
                for t in range(NT):
                    g0, g1 = int(tile_g0[t]), int(tile_g0[t + 1])
                    GT = g1 - g0
                    if L == 2:
                        idxt = off_p.tile([128, GTMAX * 8], _i16, tag="idxt")
                        nc.sync.dma_start(out=idxt[:, :GT * 8],
                                          in_=idx16d[:, g0 * 8:g1 * 8])
                        gt_all = g_p.tile([128, GTMAX * H], _bf16, tag="gt")
                        for (q, qg0, qng) in tq[t]:
                            for lg0 in range(qg0, qg0 + qng, GBMAX):
                                ng = min(GBMAX, qg0 + qng - lg0)
                                gi = nc.gpsimd.dma_gather(
                                    gt_all[:, lg0 * H:
                                           (lg0 + ng) * H].rearrange(
                                               "p (g h) -> p g h", h=H),
                                    h1tab[q * QS:(q + 1) * QS, :],
                                    idxt[:, lg0 * 8:(lg0 + ng) * 8],
                                    num_idxs=ng * 128,
                                    num_idxs_reg=ng * 128,
                                    elem_size=H,
                                    elem_step=H,
                                    single_packet=False,
                                    queue_num=q % NQUEUES)
                                if ag_deps and not dep_done[0]:
                                    for ad in ag_deps:
                                        add_dep_helper(
                                            gi.ins, ad,
                                            reason="L2 gather after AG")
                                    dep_done[0] = True
                    else:
                        feedt = feed_p.tile([128, GTMAX, F1], _bf16,
                                            tag="feed")
                        nc.sync.dma_start(
                            out=feedt[:, :GT, :],
                            in_=xsl[:, g0:g1, :])
                    keyt = misc_p.tile([128, GTMAX], _bf16, tag="keyt")
                    nc.sync.dma_start(out=keyt[:, :GT], in_=keys[:, g0:g1])
                    # one-hot for all groups of this tile in one DVE op
                    oht = oh_p.tile([128, GTMAX * COLS], _bf16, tag="oht")
                    nc.vector.tensor_tensor(
                        out=oht[:, :GT * COLS],
                        in0=bcast_mid(iota_bf[:], GT),
                        in1=bcast_inner(keyt[:, :GT], GT, COLS),
                        op=mybir.AluOpType.is_equal)

                    # aggregation PSUM: two banks of 4 windows
                    agg = [ps_agg.tile([128, 512], _f32, tag=f"agg{h}",
                                       name=f"agg{h}") for h in range(2)]
                    totals = [int(cap_wq[t * TW + wi, :].sum())
                              for wi in range(TW)]
                    emitted = [0] * TW
                    M = H if L == 2 else F1
                    for (lg, wi, q) in seqs[t]:
                        ps = agg[wi // NWPB]
                        colsl = slice((wi % NWPB) * COLS,
                                      (wi % NWPB) * COLS + COLS)
                        if L == 2:
                            lhs = gt_all[:, lg * H:(lg + 1) * H]
                        else:
                            lhs = feedt[:, lg, :]
                        nc.tensor.matmul(
                            ps[:M, colsl],
                            lhsT=lhs,
                            rhs=oht[:, lg * COLS:(lg + 1) * COLS],
                            start=(emitted[wi] == 0),
                            stop=(emitted[wi] == totals[wi] - 1))
                        emitted[wi] += 1

                    # scale (L2) / plain (L1) copy PSUM -> SBUF S~^T bf16
                    KF = H if L == 2 else F1
                    st = sb_p.tile([128, TW * COLS], _bf16, tag="st")
                    if L == 2:
                        sct = sc_p.tile([128, TW * COLS], _bf16, tag="sct")
                        nc.sync.dma_start(
                            out=sct[:], in_=screp[:, t * TW * COLS:
                                                  (t + 1) * TW * COLS])
                        for h in range(2):
                            nc.vector.tensor_tensor(
                                out=st[:, h * 512:(h + 1) * 512],
                                in0=agg[h][:, :],
                                in1=sct[:, h * 512:(h + 1) * 512],
                                op=mybir.AluOpType.mult)
                    else:
                        for h in range(2):
                            nc.vector.tensor_copy(
                                st[:KF, h * 512:(h + 1) * 512],
                                agg[h][:KF, :])

                    # transform: out2^T [128, 256 nodes]
                    op_ps = ps_out.tile([128, 256], _f32, tag="ops")
                    nsl = slice(t * 256, (t + 1) * 256)
                    nc.tensor.matmul(op_ps[:, :], lhsT=roott[:KF, :],
                                     rhs=hT_in[:KF, nsl], start=True, stop=False)
                    st3 = st[:KF, :].rearrange("p (a b) -> p a b", b=COLS)
                    for r in range(R):
                        nc.tensor.matmul(op_ps[:, :], lhsT=wts[r][:KF, :],
                                         rhs=st3[:, :, r * W:(r + 1) * W],
                                         start=False,
                                         stop=(r == R - 1))
                    # bias + relu -> hT_out (bf16)
                    nc.scalar.activation(
                        out=hT_out[:, nsl], in_=op_ps[:, :],
                        func=mybir.ActivationFunctionType.Relu,
                        bias=bt[:], scale=1.0)

                    # transpose to token-major for h1 table / pooling
                    for half in range(2):
                        tr = ps_tr.tile([128, 128], _bf16, tag="tr")
                        nc.tensor.transpose(
                            tr[:, :],
                            hT_out[:, t * 256 + half * 128:
                                   t * 256 + (half + 1) * 128],
                            ident[:])
                        tok = tok_p.tile([128, 128], _bf16, tag="tok")
                        nc.scalar.activation(
                            out=tok[:], in_=tr[:, :],
                            func=mybir.ActivationFunctionType.Copy)
                        row0 = t * 256 + half * 128
                        if L == 1:
                            nc.sync.dma_start(
                                out=h1own[row0:row0 + 128, :], in_=tok[:])
                        else:
                            bt_t = bo_p.tile([128, NGRAPH], _bf16, tag="bt")
                            nc.sync.dma_start(
                                out=bt_t[:], in_=bone[row0:row0 + 128, :])
                            pp = ps_pool.tile([NGRAPH, H], _f32, tag="pp")
                            nc.tensor.matmul(pp[:, :], lhsT=bt_t[:],
                                             rhs=tok[:], start=True, stop=True)
                            nc.vector.tensor_tensor(
                                out=pool_acc[:], in0=pool_acc[:], in1=pp[:, :],
                                op=mybir.AluOpType.add)

                    # chunked AllGather: publish h1 rows as soon as a chunk
                    # of 7 tiles is complete (overlaps remaining L1 compute)
                    if CHUNK_AG and L == 1 and (t % CHT) == CHT - 1:
                        c = t // CHT
                        ag = nc.gpsimd.collective_compute(
                            "AllGather", mybir.AluOpType.bypass,
                            replica_groups=[list(range(NCORES))],
                            ins=[h1own[c * CH:(c + 1) * CH, :]],
                            outs=[h1tab[c * NCORES * CH:
                                        (c + 1) * NCORES * CH, :]])
                        ag_insts.append(ag.ins if hasattr(ag, "ins") else ag)

            layer(1, hT_x, w1t, r1t, b1t, hT_1)
            if not CHUNK_AG:
                ag = nc.gpsimd.collective_compute(
                    "AllGather", mybir.AluOpType.bypass,
                    replica_groups=[list(range(NCORES))],
                    ins=[h1own[:, :]], outs=[h1tab[:, :]])
                ag_insts.append(ag.ins if hasattr(ag, "ins") else ag)
            # hT_x is fully consumed by layer 1 -> reuse its SBUF for h2^T
            layer(2, hT_1, w2t, r2t, b2t, hT_x, ag_deps=ag_insts)

            nc.sync.dma_start(out=pool_out[:, :], in_=pool_acc[:])

    nc.finalize()
    if legalize:
        _legalize_waits(nc)
    return nc


# ------------------------------------------------------------- runner
_CACHE = {}


def _get_compiled(cap_wq):
    key = ("nc", tuple(cap_wq.reshape(-1).tolist()))
    if key not in _CACHE:
        import jax
        from jax.sharding import Mesh, PartitionSpec
        from jax.experimental.shard_map import shard_map
        from concourse.bass2jax import (
            _bass_exec_p, partition_id_tensor, install_neuronx_cc_hook)
        install_neuronx_cc_hook()
        nc = _build_nc(cap_wq)

        partition_name = (nc.partition_id_tensor.name
                          if nc.partition_id_tensor else None)
        in_names, out_names, out_avals = [], [], []
        for alloc in nc.m.functions[0].allocations:
            if not isinstance(alloc, mybir.MemoryLocationSet):
                continue
            name = alloc.memorylocations[0].name
            if alloc.kind == "ExternalInput":
                if name != partition_name and name != (
                        nc.dbg_addr.name if nc.dbg_addr is not None else None):
                    in_names.append(name)
            elif alloc.kind == "ExternalOutput":
                out_names.append(name)
                out_avals.append(jax.core.ShapedArray(
                    tuple(alloc.tensor_shape), mybir.dt.np(alloc.dtype)))
        n_params, n_outs = len(in_names), len(out_names)
        all_in = list(in_names) + list(out_names)
        if nc.dbg_addr is not None:
            all_in.append(nc.dbg_addr.name)
        if partition_name is not None:
            all_in.append(partition_name)

        def _body(*args):
            operands = list(args)
            if nc.dbg_addr is not None:
                operands.append(jax.numpy.zeros((1, 2), jax.numpy.uint32))
            if partition_name is not None:
                operands.append(partition_id_tensor())
            outs = _bass_exec_p.bind(
                *operands, out_avals=tuple(out_avals),
                in_names=tuple(all_in), out_names=tuple(out_names),
                lowering_input_output_aliases=(),
                sim_require_finite=False, sim_require_nnan=False, nc=nc)
            return tuple(outs)

        devices = jax.devices()[:NCORES]
        mesh = Mesh(np.asarray(devices), ("core",))
        import os as _os
        donate = (() if _os.environ.get("BASS_NO_DONATE")
                  else tuple(range(n_params, n_params + n_outs)))
        sharded = jax.jit(
            shard_map(_body, mesh=mesh,
                      in_specs=(PartitionSpec("core"),) * (n_params + n_outs),
                      out_specs=(PartitionSpec("core"),) * n_outs,
                      check_rep=False),
            donate_argnums=donate,
            keep_unused=True)
        _CACHE[key] = (sharded, in_names, out_names, out_avals, mesh)
    return _CACHE[key]


def run_device(in_maps, cap_wq):
    import jax
    sharded, in_names, out_names, out_avals, mesh = _get_compiled(cap_wq)
    concat_in = [
        np.concatenate([np.asarray(in_maps[c][name]) for c in range(NCORES)],
                       axis=0)
        for name in in_names]
    concat_zeros = [
        np.zeros((NCORES * a.shape[0], *a.shape[1:]), a.dtype)
        for a in out_avals]
    out_arrs = sharded(*concat_in, *concat_zeros)
    jax.block_until_ready(out_arrs)
    res = [
        {name: np.asarray(out_arrs[i]).reshape(NCORES, *out_avals[i].shape)[c]
         for i, name in enumerate(out_names)}
        for c in range(NCORES)]
    return res


def kernel(x, W1, root1, b1, W2, root2, b2, edge_index, edge_type, batch):
    in_maps, hc = _host_prep(x, W1, root1, b1, W2, root2, b2,
                             edge_index, edge_type, batch)
    res = run_device(in_maps, hc["cap_wq"])
    total = np.zeros((NGRAPH, H), dtype=np.float32)
    for k in range(NCORES):
        total += res[k]["pool_out"]
    return (total / hc["gcounts"][:, None]).astype(np.float32)
